# revision 25
# baseline (speedup 1.0000x reference)
"""Self-contained Trainium2 Bass kernel for the EnhancedMambaFusion model.

Strategy: pure data-parallel over 8 NeuronCores (batch 8192 -> 1024/core).
On-device everything is computed feature-major ([feature_chunk(128p), batch])
so no activation transposes are needed between layers; weights are
host-transposed to [din, dout] and cast to bf16 (fp32 PSUM accumulation).
Host-side algebraic folding (exact up to fp32 rounding):
  - depthwise-conv last tap scale folded into the in_proj x-half
  - MHA over seq-len 1 is linear: v-proj @ out-proj @ gate1 collapsed into
    a single 2048->1024 matmul feeding the gate softmax
"""

import sys
import threading
import types

sys.path.insert(0, "/opt/trn_rl_repo")

import numpy as np
import ml_dtypes

import concourse.bass as bass
import concourse.bacc as bacc
import concourse.tile as tile
import concourse.mybir as mybir
from concourse.masks import make_identity

# The greedy ACT-table chooser pairs Exp with "exp_and_others" (no Ln) and Ln
# with "natural_log" (no Exp), reloading the LUT between every softplus pair.
# Table *ids* are positional, so keep the list order/length but blank the
# membership of every table except the two this kernel wants; the chooser then
# lands on natural_log_exp_and_others (Exp+Ln) and silu_and_others (Silu) and
# switches only at phase boundaries.
import concourse.hw_specs as _hw_specs
_orig_get_tables = _hw_specs.get_activation_tables

def _patched_tables(arch):
    t = _orig_get_tables(arch)
    keep = {"natural_log_exp_and_others", "silu_and_others"}
    unk = {mybir.ActivationFunctionType.Unknown}
    return {n: (fns if n in keep else unk) for n, fns in t.items()}

bacc.get_activation_tables = _patched_tables

BF16NP = ml_dtypes.bfloat16
BF = mybir.dt.bfloat16
F32 = mybir.dt.float32
AF = mybir.ActivationFunctionType
OP = mybir.AluOpType

N_CORES = 8
B = 8192
DIM = 1024
D_MODEL = 2048
D_INNER = 4096
DT_RANK = 128
D_STATE = 16
OUT = 256
P = 128
BC = B // N_CORES   # batch per core
BH = 512            # half-batch = matmul moving width
NH = BC // BH
EPS = 1e-5

KD = DIM // P        # 8   feature chunks of 1024
KM = D_MODEL // P    # 16  feature chunks of 2048
KI = D_INNER // P    # 32  feature chunks of 4096
KO = OUT // P        # 2   feature chunks of 256


# ---------------------------------------------------------------- host prep

def _t(a):
    """weight [dout, din] fp32 -> bf16 [din, dout] contiguous"""
    return np.ascontiguousarray(np.asarray(a, np.float32).astype(BF16NP).T)


def _vp(v):
    """per-feature vector [d] -> fp32 [128, d/128] (f = ko*128 + p)"""
    v = np.asarray(v, np.float32)
    return np.ascontiguousarray(v.reshape(-1, P).T)


def _prep_weights(i):
    w = {}
    w["w_img"] = _t(i["img_w"])
    w["w_txt"] = _t(i["txt_w"])
    w["b_img"] = _vp(i["img_b"]); w["g_img"] = _vp(i["img_g"]); w["be_img"] = _vp(i["img_beta"])
    w["b_txt"] = _vp(i["txt_b"]); w["g_txt"] = _vp(i["txt_g"]); w["be_txt"] = _vp(i["txt_beta"])
    for l in range(2):
        cw = np.asarray(i["m_conv_w"], np.float32)[l, :, -1]          # [4096] last tap
        win = np.asarray(i["m_in_w"], np.float32)[l]                  # [8192, 2048]
        w[f"w_inx{l}"] = _t(win[:D_INNER] * cw[:, None])              # [2048, 4096]
        w[f"w_inz{l}"] = _t(win[D_INNER:])                            # [2048, 4096]
        w[f"b_inx{l}"] = _vp(np.asarray(i["m_conv_b"], np.float32)[l])
        # x_proj weights: [4096, 192] = [dt_rank(128) | B(16)@128 pad | C(16)@160 pad]
        # so the B/C matmul output lands at 32-aligned partition bases.
        wxT = np.asarray(i["m_x_w"], np.float32)[l].T                 # [4096, 160]
        wx2 = np.zeros((D_INNER, 192), np.float32)
        wx2[:, 0:DT_RANK] = wxT[:, 0:DT_RANK]
        wx2[:, 128:128 + D_STATE] = wxT[:, DT_RANK:DT_RANK + D_STATE]
        wx2[:, 160:160 + D_STATE] = wxT[:, DT_RANK + D_STATE:]
        w[f"w_x{l}"] = np.ascontiguousarray(wx2.astype(BF16NP))
        w[f"w_dt{l}"] = _t(np.asarray(i["m_dt_w"], np.float32)[l])    # [128, 4096]
        w[f"b_dt{l}"] = _vp(np.asarray(i["m_dt_b"], np.float32)[l])
        w[f"dvec{l}"] = _vp(np.asarray(i["m_D"], np.float32)[l])
        w[f"w_out{l}"] = _t(np.asarray(i["m_out_w"], np.float32)[l])  # [4096, 2048]
        w[f"g_ln{l}"] = _vp(np.asarray(i["m_ln_g"], np.float32)[l])
        w[f"b_ln{l}"] = _vp(np.asarray(i["m_ln_b"], np.float32)[l])
    # Fold each h-producing LN's beta into its consumers (h stores the
    # unshifted core); betas: b01 after img/txt, m_ln_b[l] after layer l.
    b01 = np.concatenate([np.asarray(i["img_beta"], np.float32),
                          np.asarray(i["txt_beta"], np.float32)])
    bprev = {0: b01, 1: np.asarray(i["m_ln_g"], np.float32)[0] * 0
             + np.asarray(i["m_ln_b"], np.float32)[0]}
    for l in range(2):
        w[f"b_inx{l}"] = _vp(np.asarray(i["m_conv_b"], np.float32)[l]
                             + bprev[l] @ np.asarray(w[f"w_inx{l}"], np.float32))
        w[f"bz{l}"] = _vp(bprev[l] @ np.asarray(w[f"w_inz{l}"], np.float32))
        w[f"bres{l}"] = _vp(bprev[l])
    bL1 = np.asarray(i["m_ln_b"], np.float32)[1]
    # MHA(seq=1) + gate1 collapse: attn_out = h @ (Wo Wv).T + (Wo bv + bo)
    wv = np.asarray(i["attn_in_w"], np.float32)[2 * D_MODEL:]
    bv = np.asarray(i["attn_in_b"], np.float32)[2 * D_MODEL:]
    wo = np.asarray(i["attn_out_w"], np.float32)
    bo = np.asarray(i["attn_out_b"], np.float32)
    wvo = wo @ wv
    bvo = wo @ bv + bo
    g1w = np.asarray(i["gate1_w"], np.float32)
    wg = g1w @ wvo                                                    # [1024, 2048]
    bg = g1w @ bvo + np.asarray(i["gate1_b"], np.float32)
    w["w_gate"] = _t(wg)
    w["b_gate"] = _vp(bg + bL1 @ np.asarray(w["w_gate"], np.float32))
    w["w_g2"] = _t(i["gate2_w"])                                      # [1024, 2]
    w["b_g2"] = np.ascontiguousarray(np.asarray(i["gate2_b"], np.float32).reshape(2, 1))
    w["pm1"] = np.asarray([[1.0], [-1.0]], np.float32)
    w["w_fc"] = _t(i["fc_w"])                                         # [2048, 256]
    w["b_fc"] = _vp(np.asarray(i["fc_b"], np.float32)
                    + bL1 @ np.asarray(w["w_fc"], np.float32)); w["g_fin"] = _vp(i["fin_g"]); w["b_fin"] = _vp(i["fin_b"])
    return w


# ---------------------------------------------------------------- device program

def _emit(ctx, nc, tc, D):
    """Emit the full network under a TileContext. D maps names -> dram APs."""

    def pool(name, bufs, space="SBUF"):
        return ctx.enter_context(tc.tile_pool(name=name, bufs=bufs, space=space))

    consts = pool("consts", 1)
    wt = pool("wt", 4)            # streamed weight tiles
    wres = pool("wres", 1)        # per-layer resident weights
    acts = pool("acts", 2)        # h
    bigp = pool("bigp", 1)        # xp
    medp = pool("medp", 1)        # pre / gh (shared slot)
    inp = pool("inp", 1)          # input feature tiles
    sm = pool("sm", 3)            # small rotating temps (inner-loop)
    szp = pool("szp", 6)          # silu(z) chunks (deferred-consumer depth)
    sm1 = pool("sm1", 1)          # small single-shot temps
    sm2 = pool("sm2", 2)          # output staging
    smc = pool("smc", 4)          # broadcast rows etc.
    psm = pool("psm", 3, space="PSUM")
    psd_p = pool("psd", 2, space="PSUM")
    pss = pool("pss", 2, space="PSUM")
    pst = pool("pst", 1, space="PSUM")

    # constants
    ones_b = consts.tile([P, 1], BF, tag="ones_b")
    nc.vector.memset(ones_b, 1.0)
    ones_f = consts.tile([P, 1], F32, tag="ones_f")
    nc.vector.memset(ones_f, 1.0)
    ident = consts.tile([P, P], F32, tag="ident")
    make_identity(nc, ident)
    eps_t = consts.tile([1, 1], F32, tag="eps")
    nc.vector.memset(eps_t, EPS)

    def cvec(name, cols):
        t = consts.tile([P, cols], F32, tag=name)
        nc.sync.dma_start(t, D[name][:, :])
        return t

    b_img = cvec("b_img", KD); g_img = cvec("g_img", KD); be_img = cvec("be_img", KD)
    b_txt = cvec("b_txt", KD); g_txt = cvec("g_txt", KD); be_txt = cvec("be_txt", KD)
    b_inx = [cvec(f"b_inx{l}", KI) for l in range(2)]
    b_dt = [cvec(f"b_dt{l}", KI) for l in range(2)]
    dvec = [cvec(f"dvec{l}", KI) for l in range(2)]
    g_ln = [cvec(f"g_ln{l}", KM) for l in range(2)]
    b_ln = [cvec(f"b_ln{l}", KM) for l in range(2)]
    bz = [cvec(f"bz{l}", KI) for l in range(2)]
    bres = [cvec(f"bres{l}", KM) for l in range(2)]

    def neg_of(t, cols):
        n = consts.tile([P, cols], F32, tag=f"neg_{t.name}")
        nc.vector.tensor_scalar_mul(n, t, -1.0)
        return n

    ng_img = neg_of(g_img, KD); ng_txt = neg_of(g_txt, KD)
    ng_ln = [neg_of(g_ln[l], KM) for l in range(2)]
    b_gate = cvec("b_gate", KD)
    b_fc = cvec("b_fc", KO); g_fin = cvec("g_fin", KO); b_fin = cvec("b_fin", KO)
    b_g2 = consts.tile([2, 1], F32, tag="b_g2")
    nc.sync.dma_start(b_g2, D["b_g2"][:, :])
    pm1 = consts.tile([2, 1], F32, tag="pm1")
    nc.sync.dma_start(pm1, D["pm1"][:, :])

    # resident small weights
    wg2 = consts.tile([P, KD, 2], BF, tag="wg2")
    nc.sync.dma_start(wg2, D["w_g2"].rearrange("(ko p) n -> p ko n", p=P))
    wfc = consts.tile([P, KM, OUT], BF, tag="wfc")
    nc.sync.dma_start(wfc, D["w_fc"].rearrange("(ko p) n -> p ko n", p=P))

    def linear_prefetch(wname, K, M, mtile=2, ktile=16, groups=1):
        """Pre-issue the weight DMAs for the first `groups` m-groups."""
        wr = D[wname].rearrange("(ko p) n -> p ko n", p=P)
        out = []
        for m0 in range(0, min(M, groups * mtile), mtile):
            mt = min(mtile, M - m0)
            tiles = []
            for k0 in range(0, K, ktile):
                kt = min(ktile, K - k0)
                t = wt.tile([P, kt, mt * P], BF, tag="wt")
                nc.sync.dma_start(t, wr[:, k0:k0 + kt, m0 * P:(m0 + mt) * P])
                tiles.append((k0, kt, t))
            out.append(tiles)
        return out

    def linear(wname, K, M, rhs_fn, out_fn, mtile=2, ktile=16, prefetched=None):
        """out[m] = sum_k W[k,m].T @ rhs[k]; streams weight tiles from DRAM."""
        wr = D[wname].rearrange("(ko p) n -> p ko n", p=P)
        for gi, m0 in enumerate(range(0, M, mtile)):
            mt = min(mtile, M - m0)
            if prefetched is not None and gi < len(prefetched):
                tiles = prefetched[gi]
            else:
                tiles = []
                for k0 in range(0, K, ktile):
                    kt = min(ktile, K - k0)
                    t = wt.tile([P, kt, mt * P], BF, tag="wt")
                    nc.sync.dma_start(t, wr[:, k0:k0 + kt, m0 * P:(m0 + mt) * P])
                    tiles.append((k0, kt, t))
            for m in range(m0, m0 + mt):
                ps = psm.tile([P, BH], F32, tag="mm")
                ki = 0
                for (k0, kt, t) in tiles:
                    for kk in range(kt):
                        nc.tensor.matmul(
                            ps, t[:, kk, (m - m0) * P:(m - m0 + 1) * P], rhs_fn(k0 + kk),
                            start=(ki == 0), stop=(ki == K - 1))
                        ki += 1
                out_fn(m, ps)

    # --- layernorm split: stats accumulate inline with the producing loop ---
    def ln_begin():
        ps_s = pss.tile([1, BH], F32, tag="st")
        ps_q = pss.tile([1, BH], F32, tag="st")
        return (ps_s, ps_q)

    def ln_add(st, chunk, k, K, fp32=False):
        ps_s, ps_q = st
        ones = ones_f if fp32 else ones_b
        nc.tensor.matmul(ps_s, ones, chunk, start=(k == 0), stop=(k == K - 1))
        sq = sm.tile([P, BH], F32 if fp32 else BF, tag="sqk")
        nc.scalar.activation(sq, chunk, AF.Square)
        nc.tensor.matmul(ps_q, ones, sq, start=(k == 0), stop=(k == K - 1))

    def ln_apply(st, src, K, g_t, be_t, dst, d, ng_t=None):
        """rstd via exp(-0.5*ln(var+eps)) so everything stays on one ACT table."""
        ps_s, ps_q = st
        sums = smc.tile([1, BH], F32, tag="r1")
        nc.scalar.copy(sums, ps_s)
        musq = smc.tile([1, BH], F32, tag="r1")
        nc.vector.scalar_tensor_tensor(musq, sums, 1.0 / (d * d), sums,
                                       OP.mult, OP.mult)
        var = smc.tile([1, BH], F32, tag="r1")
        nc.vector.scalar_tensor_tensor(var, ps_q, 1.0 / d, musq,
                                       OP.mult, OP.subtract)
        lv = smc.tile([1, BH], F32, tag="r1")
        nc.scalar.activation(lv, var, AF.Ln, bias=eps_t)
        rstd = smc.tile([1, BH], F32, tag="r1")
        nc.scalar.activation(rstd, lv, AF.Exp, scale=-0.5)
        tmt = smc.tile([1, BH], F32, tag="r1")
        nc.vector.scalar_tensor_tensor(tmt, sums, 1.0 / d, rstd,
                                       OP.mult, OP.mult)
        S = smc.tile([P, BH], F32, tag="bcast")
        nc.gpsimd.partition_broadcast(S, rstd)
        T = smc.tile([P, BH], F32, tag="bcast")
        nc.gpsimd.partition_broadcast(T, tmt)
        if ng_t is not None:
            # beta folded into consumers: dst = x*(S*g) - T*g, two fused passes
            for k in range(K):
                tmp = sm.tile([P, BH], F32, tag="f32tmp")
                nc.vector.scalar_tensor_tensor(tmp, S, g_t[:, k:k + 1],
                                               src[:, k, :], OP.mult, OP.mult)
                nc.vector.scalar_tensor_tensor(dst[:, k, :], T, ng_t[:, k:k + 1],
                                               tmp, OP.mult, OP.add)
        else:
            for k in range(K):
                tmp = sm.tile([P, BH], F32, tag="f32tmp")
                nc.vector.tensor_tensor(tmp, src[:, k, :], S, OP.mult)
                nc.vector.tensor_tensor(tmp, tmp, T, OP.subtract)
                nc.scalar.activation(dst[:, k, :], tmp, AF.Identity,
                                     bias=be_t[:, k:k + 1], scale=g_t[:, k:k + 1])

    for hh in range(NH):
        bsl = slice(hh * BH, (hh + 1) * BH)

        # ---- input projections -> h [P, KM, BH]
        h_t = acts.tile([P, KM, BH], BF, tag="h")
        for (srcn, wname, bvec, gvec, bevec, ngvec, off) in (
            ("imgT", "w_img", b_img, g_img, be_img, ng_img, 0),
            ("txtT", "w_txt", b_txt, g_txt, be_txt, ng_txt, KD),
        ):
            x_in = inp.tile([P, KD, BH], BF, tag="inp")
            nc.sync.dma_start(x_in, D[srcn].rearrange("(ko p) b -> p ko b", p=P)[:, :, bsl])
            prj = medp.tile([P, KD, BH], BF, tag="pre")
            st = ln_begin()

            def prj_out(m, ps, prj=prj, bvec=bvec, st=st):
                nc.scalar.activation(prj[:, m, :], ps, AF.Relu, bias=bvec[:, m:m + 1])
                ln_add(st, prj[:, m, :], m, KD)

            linear(wname, KD, KD,
                   rhs_fn=lambda k, x_in=x_in: x_in[:, k, :],
                   out_fn=prj_out)
            ln_apply(st, prj, KD, gvec, bevec, h_t[:, off:off + KD, :], DIM,
                     ng_t=ngvec)

        # ---- two mamba blocks
        for l in range(2):
            # x' = silu(h @ w_inx + conv_b)  [P, KI, BH]  (native Silu table)
            xp = bigp.tile([P, KI, BH], BF, tag="xp")
            linear(f"w_inx{l}", KM, KI,
                   rhs_fn=lambda k, h_t=h_t: h_t[:, k, :],
                   out_fn=lambda m, ps, xp=xp, l=l: nc.scalar.activation(
                       xp[:, m, :], ps, AF.Silu, bias=b_inx[l][:, m:m + 1]))

            # x_proj m0 -> dblr; dt matmuls for chunks 0..15 interleaved into
            # the m1 (B/C) chain so the ACT softplus (exp/ln) drain never
            # stalls the PE.
            wx = wres.tile([P, KI, 192], BF, tag="wx")
            nc.sync.dma_start(wx, D[f"w_x{l}"].rearrange("(ko p) n -> p ko n", p=P))
            wdt = wres.tile([P, D_INNER], BF, tag="wdt")
            nc.sync.dma_start(wdt, D[f"w_dt{l}"][:, :])
            ps0 = psm.tile([P, BH], F32, tag="mm")
            for k in range(KI):
                nc.tensor.matmul(ps0, wx[:, k, 0:DT_RANK], xp[:, k, :],
                                 start=(k == 0), stop=(k == KI - 1))
            dblr = smc.tile([P, BH], BF, tag="dblr")
            nc.vector.tensor_copy(dblr, ps0)

            def dt_mm(dtg, i, k):
                psd = psd_p.tile([P, BH], F32, tag="psd")
                nc.tensor.matmul(psd, wdt[:, k * P:(k + 1) * P], dblr,
                                 start=True, stop=True)
                ex = sm.tile([P, BH], F32, tag="f32tmp")
                nc.scalar.activation(ex, psd, AF.Exp, bias=b_dt[l][:, k:k + 1])
                nc.scalar.activation(dtg[:, i, :], ex, AF.Ln, bias=ones_f)

            dtg1 = medp.tile([P, KM, BH], BF, tag="pre")
            ps1 = pss.tile([64, BH], F32, tag="st")
            for i in range(KM):
                dt_mm(dtg1, i, i)
                for kk in (2 * i, 2 * i + 1):
                    nc.tensor.matmul(ps1, wx[:, kk, 128:192], xp[:, kk, :],
                                     start=(kk == 0), stop=(kk == KI - 1))
            bcb = sm1.tile([D_STATE, BH], F32, tag="bcb")
            nc.scalar.copy(bcb, ps1[0:D_STATE])
            bcc = sm1.tile([D_STATE, BH], F32, tag="bcc")
            nc.scalar.copy(bcc, ps1[32:32 + D_STATE])
            prod = sm1.tile([D_STATE, BH], F32, tag="prod")
            nc.vector.tensor_tensor(prod, bcb, bcc, OP.mult)
            ps_bc = pss.tile([1, BH], F32, tag="st")
            nc.tensor.matmul(ps_bc, ones_f[:D_STATE], prod, start=True, stop=True)
            bc1 = smc.tile([1, BH], F32, tag="r1")
            nc.scalar.copy(bc1, ps_bc)
            BCt = smc.tile([P, BH], F32, tag="bcast")
            nc.gpsimd.partition_broadcast(BCt, bc1)

            # fused y loop: y = x' * (dt*bc + D) * silu(z), native Silu table.
            wzr = D[f"w_inz{l}"].rearrange("(ko p) n -> p ko n", p=P)

            def z_part(k, kk_in_pair, wz_t):
                psz = psm.tile([P, BH], F32, tag="mm")
                for j in range(KM):
                    nc.tensor.matmul(psz, wz_t[:, j, kk_in_pair * P:(kk_in_pair + 1) * P],
                                     h_t[:, j, :],
                                     start=(j == 0), stop=(j == KM - 1))
                sz = szp.tile([P, BH], BF, tag="sz")
                nc.scalar.activation(sz, psz, AF.Silu, bias=bz[l][:, k:k + 1])
                return sz

            def y_part(k, dtg, i, sz):
                u = sm.tile([P, BH], F32, tag="f32tmp")
                nc.vector.tensor_tensor(u, dtg[:, i, :], BCt, OP.mult)
                nc.vector.scalar_tensor_tensor(u, u, dvec[l][:, k:k + 1],
                                               xp[:, k, :], OP.add, OP.mult)
                nc.vector.tensor_tensor(xp[:, k, :], u, sz, OP.mult)

            def wz_tile(k0):
                t = wt.tile([P, KM, 2 * P], BF, tag="wt")
                nc.sync.dma_start(t, wzr[:, :, k0 * P:(k0 + 2) * P])
                return t

            for k0 in range(0, KM, 2):           # B1: chunks 0..15
                t = wz_tile(k0)
                for kk in range(2):
                    k = k0 + kk
                    y_part(k, dtg1, k, z_part(k, kk, t))
            deferred = []
            for k0 in range(KM, KM + 4, 2):      # B2 head: z only, y deferred
                t = wz_tile(k0)
                for kk in range(2):
                    k = k0 + kk
                    deferred.append((k, z_part(k, kk, t)))
            dtg2 = medp.tile([P, KM, BH], BF, tag="pre")
            for i in range(KM):                  # A2 block
                dt_mm(dtg2, i, KM + i)
            for (k, sz) in deferred:
                y_part(k, dtg2, k - KM, sz)
            wo_pref = linear_prefetch(f"w_out{l}", KI, KM, mtile=1, ktile=KI, groups=2)
            for k0 in range(KM + 4, KI, 2):      # B2 tail
                t = wz_tile(k0)
                for kk in range(2):
                    k = k0 + kk
                    y_part(k, dtg2, k - KM, z_part(k, kk, t))

            # out_proj + residual -> pre (stats inline), then LN -> h
            pre = medp.tile([P, KM, BH], BF, tag="pre")
            st = ln_begin()

            def pre_out(m, ps, pre=pre, h_t=h_t, st=st, l=l):
                nc.vector.scalar_tensor_tensor(pre[:, m, :], h_t[:, m, :],
                                               bres[l][:, m:m + 1], ps,
                                               OP.add, OP.add)
                ln_add(st, pre[:, m, :], m, KM)

            linear(f"w_out{l}", KI, KM,
                   rhs_fn=lambda k, xp=xp: xp[:, k, :],
                   out_fn=pre_out, mtile=1, ktile=KI, prefetched=wo_pref)
            ln_apply(st, pre, KM, g_ln[l], b_ln[l], h_t, D_MODEL, ng_t=ng_ln[l])

        # ---- gate head (folded attn) -> gate weights [BH, 2]
        gh = medp.tile([P, KD, BH], BF, tag="pre")
        linear("w_gate", KM, KD,
               rhs_fn=lambda k, h_t=h_t: h_t[:, k, :],
               out_fn=lambda m, ps, gh=gh: nc.scalar.activation(
                   gh[:, m, :], ps, AF.Relu, bias=b_gate[:, m:m + 1]))
        ps_lg = pss.tile([2, BH], F32, tag="st")
        for k in range(KD):
            nc.tensor.matmul(ps_lg, wg2[:, k, :], gh[:, k, :],
                             start=(k == 0), stop=(k == KD - 1))
        lg = sm1.tile([2, BH], F32, tag="lg")
        nc.scalar.activation(lg, ps_lg, AF.Identity, bias=b_g2)
        ps_d = pss.tile([1, BH], F32, tag="st")
        nc.tensor.matmul(ps_d, pm1, lg, start=True, stop=True)
        # sigmoid via exp: gw0 = 1/(1+exp(-dd))
        egw = smc.tile([1, BH], F32, tag="r1")
        nc.scalar.activation(egw, ps_d, AF.Exp, scale=-1.0)
        nc.vector.tensor_scalar(egw, egw, 1.0, None, OP.add)
        gw0 = smc.tile([1, BH], F32, tag="r1")
        nc.vector.reciprocal(gw0, egw)
        gw1 = smc.tile([1, BH], F32, tag="r1")
        nc.vector.tensor_scalar(gw1, gw0, -1.0, 1.0, OP.mult, OP.add)
        nc.sync.dma_start(D["gw"][bsl, 0:1].rearrange("b t -> t b"), gw0)
        nc.sync.dma_start(D["gw"][bsl, 1:2].rearrange("b t -> t b"), gw1)

        # ---- fc head + final LN -> fused [BH, 256]
        pf = sm1.tile([P, KO, BH], F32, tag="pf")
        st = ln_begin()
        for m in range(KO):
            ps = psm.tile([P, BH], F32, tag="mm")
            for k in range(KM):
                nc.tensor.matmul(ps, wfc[:, k, m * P:(m + 1) * P], h_t[:, k, :],
                                 start=(k == 0), stop=(k == KM - 1))
            nc.scalar.activation(pf[:, m, :], ps, AF.Identity, bias=b_fc[:, m:m + 1])
            ln_add(st, pf[:, m, :], m, KO, fp32=True)
        ff = sm1.tile([P, KO, BH], F32, tag="ff")
        ln_apply(st, pf, KO, g_fin, b_fin, ff, OUT)
        for b4 in range(BH // P):
            ob = sm2.tile([P, OUT], F32, tag="ob")
            for m in range(KO):
                p_t = pst.tile([P, P], F32, tag="tp")
                nc.tensor.transpose(p_t, ff[:, m, b4 * P:(b4 + 1) * P], ident)
                nc.vector.tensor_copy(ob[:, m * P:(m + 1) * P], p_t)
            nc.sync.dma_start(D["fused"][hh * BH + b4 * P: hh * BH + (b4 + 1) * P, :], ob)


# ---------------------------------------------------------------- build + run

_CACHE = {}
_LOCK = threading.Lock()


def _get_program():
    with _LOCK:
        if "nc" in _CACHE:
            return _CACHE["nc"]
        nc = bacc.Bacc("TRN2", target_bir_lowering=False, debug=False,
                       num_devices=N_CORES)
        D = {}

        def din(name, shape, dt):
            D[name] = nc.dram_tensor(name, shape, dt, kind="ExternalInput").ap()

        din("imgT", (DIM, BC), BF)
        din("txtT", (DIM, BC), BF)
        din("w_img", (DIM, DIM), BF)
        din("w_txt", (DIM, DIM), BF)
        for n in ("b_img", "g_img", "be_img", "b_txt", "g_txt", "be_txt", "b_gate"):
            din(n, (P, KD), F32)
        for l in range(2):
            din(f"w_inx{l}", (D_MODEL, D_INNER), BF)
            din(f"w_inz{l}", (D_MODEL, D_INNER), BF)
            din(f"w_x{l}", (D_INNER, 192), BF)
            din(f"w_dt{l}", (DT_RANK, D_INNER), BF)
            din(f"w_out{l}", (D_INNER, D_MODEL), BF)
            for n in (f"b_inx{l}", f"b_dt{l}", f"dvec{l}", f"bz{l}"):
                din(n, (P, KI), F32)
            for n in (f"g_ln{l}", f"b_ln{l}", f"bres{l}"):
                din(n, (P, KM), F32)
        din("w_gate", (D_MODEL, DIM), BF)
        din("w_g2", (DIM, 2), BF)
        din("b_g2", (2, 1), F32)
        din("pm1", (2, 1), F32)
        din("w_fc", (D_MODEL, OUT), BF)
        for n in ("b_fc", "g_fin", "b_fin"):
            din(n, (P, KO), F32)
        D["fused"] = nc.dram_tensor("fused", (BC, OUT), F32, kind="ExternalOutput").ap()
        D["gw"] = nc.dram_tensor("gw", (BC, 2), F32, kind="ExternalOutput").ap()

        from contextlib import ExitStack
        with tile.TileContext(nc) as tc:
            with ExitStack() as es:
                _emit(es, nc, tc, D)
        nc.compile()
        _CACHE["nc"] = nc
        return nc


def _install_ntff_hook():
    """trace=True under axon needs antenv.axon_hooks, absent in this image."""
    if "antenv.axon_hooks" in sys.modules:
        return
    try:
        from trn_agent_boot.trn_boot import _ntff_profile_via_ctypes
        hook = _ntff_profile_via_ctypes("/opt/axon/libaxon_pjrt.so")
    except Exception:
        hook = None
    mod = types.ModuleType("antenv.axon_hooks")
    mod.get_axon_ntff_profile_hook = lambda: hook
    sys.modules["antenv.axon_hooks"] = mod


def kernel(_trace=False, **inputs):
    from concourse.bass_utils import run_bass_kernel_spmd

    _install_ntff_hook()
    nc = _get_program()
    w = _prep_weights(inputs)
    imgT = np.ascontiguousarray(
        np.asarray(inputs["image_features"], np.float32).astype(BF16NP).T)  # [DIM, B]
    txtT = np.ascontiguousarray(
        np.asarray(inputs["text_features"], np.float32).astype(BF16NP).T)
    in_maps = []
    for c in range(N_CORES):
        m = dict(w)
        m["imgT"] = np.ascontiguousarray(imgT[:, c * BC:(c + 1) * BC])
        m["txtT"] = np.ascontiguousarray(txtT[:, c * BC:(c + 1) * BC])
        in_maps.append(m)
    res = run_bass_kernel_spmd(nc, in_maps, core_ids=list(range(N_CORES)),
                               trace=_trace)
    fused = np.concatenate([res.results[c]["fused"] for c in range(N_CORES)], axis=0)
    gw = np.concatenate([res.results[c]["gw"] for c in range(N_CORES)], axis=0)
    if _trace:
        kernel.last_exec_time_ns = res.exec_time_ns
        kernel.last_results = res
    return fused, gw


# revision 27
# speedup vs baseline: 1.0001x; 1.0001x over previous
"""Self-contained Trainium2 Bass kernel for the EnhancedMambaFusion model.

Strategy: pure data-parallel over 8 NeuronCores (batch 8192 -> 1024/core).
On-device everything is computed feature-major ([feature_chunk(128p), batch])
so no activation transposes are needed between layers; weights are
host-transposed to [din, dout] and cast to bf16 (fp32 PSUM accumulation).
Host-side algebraic folding (exact up to fp32 rounding):
  - depthwise-conv last tap scale folded into the in_proj x-half
  - MHA over seq-len 1 is linear: v-proj @ out-proj @ gate1 collapsed into
    a single 2048->1024 matmul feeding the gate softmax
"""

import sys
import threading
import types

sys.path.insert(0, "/opt/trn_rl_repo")

import numpy as np
import ml_dtypes

import concourse.bass as bass
import concourse.bacc as bacc
import concourse.tile as tile
import concourse.mybir as mybir
from concourse.masks import make_identity

# The greedy ACT-table chooser pairs Exp with "exp_and_others" (no Ln) and Ln
# with "natural_log" (no Exp), reloading the LUT between every softplus pair.
# Table *ids* are positional, so keep the list order/length but blank the
# membership of every table except the two this kernel wants; the chooser then
# lands on natural_log_exp_and_others (Exp+Ln) and silu_and_others (Silu) and
# switches only at phase boundaries.
import concourse.hw_specs as _hw_specs
_orig_get_tables = _hw_specs.get_activation_tables

def _patched_tables(arch):
    t = _orig_get_tables(arch)
    keep = {"natural_log_exp_and_others", "silu_and_others"}
    unk = {mybir.ActivationFunctionType.Unknown}
    return {n: (fns if n in keep else unk) for n, fns in t.items()}

bacc.get_activation_tables = _patched_tables

BF16NP = ml_dtypes.bfloat16
BF = mybir.dt.bfloat16
F32 = mybir.dt.float32
AF = mybir.ActivationFunctionType
OP = mybir.AluOpType

N_CORES = 8
B = 8192
DIM = 1024
D_MODEL = 2048
D_INNER = 4096
DT_RANK = 128
D_STATE = 16
OUT = 256
P = 128
BC = B // N_CORES   # batch per core
BH = 512            # half-batch = matmul moving width
NH = BC // BH
EPS = 1e-5

KD = DIM // P        # 8   feature chunks of 1024
KM = D_MODEL // P    # 16  feature chunks of 2048
KI = D_INNER // P    # 32  feature chunks of 4096
KO = OUT // P        # 2   feature chunks of 256


# ---------------------------------------------------------------- host prep

def _t(a):
    """weight [dout, din] fp32 -> bf16 [din, dout] contiguous"""
    return np.ascontiguousarray(np.asarray(a, np.float32).astype(BF16NP).T)


def _vp(v):
    """per-feature vector [d] -> fp32 [128, d/128] (f = ko*128 + p)"""
    v = np.asarray(v, np.float32)
    return np.ascontiguousarray(v.reshape(-1, P).T)


def _prep_weights(i):
    w = {}
    w["w_img"] = _t(i["img_w"])
    w["w_txt"] = _t(i["txt_w"])
    w["b_img"] = _vp(i["img_b"]); w["g_img"] = _vp(i["img_g"]); w["be_img"] = _vp(i["img_beta"])
    w["b_txt"] = _vp(i["txt_b"]); w["g_txt"] = _vp(i["txt_g"]); w["be_txt"] = _vp(i["txt_beta"])
    for l in range(2):
        cw = np.asarray(i["m_conv_w"], np.float32)[l, :, -1]          # [4096] last tap
        win = np.asarray(i["m_in_w"], np.float32)[l]                  # [8192, 2048]
        w[f"w_inx{l}"] = _t(win[:D_INNER] * cw[:, None])              # [2048, 4096]
        w[f"w_inz{l}"] = _t(win[D_INNER:])                            # [2048, 4096]
        w[f"b_inx{l}"] = _vp(np.asarray(i["m_conv_b"], np.float32)[l])
        # x_proj weights: [4096, 192] = [dt_rank(128) | B(16)@128 pad | C(16)@160 pad]
        # so the B/C matmul output lands at 32-aligned partition bases.
        wxT = np.asarray(i["m_x_w"], np.float32)[l].T                 # [4096, 160]
        wx2 = np.zeros((D_INNER, 192), np.float32)
        wx2[:, 0:DT_RANK] = wxT[:, 0:DT_RANK]
        wx2[:, 128:128 + D_STATE] = wxT[:, DT_RANK:DT_RANK + D_STATE]
        wx2[:, 160:160 + D_STATE] = wxT[:, DT_RANK + D_STATE:]
        w[f"w_x{l}"] = np.ascontiguousarray(wx2.astype(BF16NP))
        w[f"w_dt{l}"] = _t(np.asarray(i["m_dt_w"], np.float32)[l])    # [128, 4096]
        w[f"b_dt{l}"] = _vp(np.asarray(i["m_dt_b"], np.float32)[l])
        w[f"dvec{l}"] = _vp(np.asarray(i["m_D"], np.float32)[l])
        w[f"w_out{l}"] = _t(np.asarray(i["m_out_w"], np.float32)[l])  # [4096, 2048]
        w[f"g_ln{l}"] = _vp(np.asarray(i["m_ln_g"], np.float32)[l])
        w[f"b_ln{l}"] = _vp(np.asarray(i["m_ln_b"], np.float32)[l])
    # Fold each h-producing LN's beta into its consumers (h stores the
    # unshifted core); betas: b01 after img/txt, m_ln_b[l] after layer l.
    b01 = np.concatenate([np.asarray(i["img_beta"], np.float32),
                          np.asarray(i["txt_beta"], np.float32)])
    bprev = {0: b01, 1: np.asarray(i["m_ln_g"], np.float32)[0] * 0
             + np.asarray(i["m_ln_b"], np.float32)[0]}
    for l in range(2):
        w[f"b_inx{l}"] = _vp(np.asarray(i["m_conv_b"], np.float32)[l]
                             + bprev[l] @ np.asarray(w[f"w_inx{l}"], np.float32))
        w[f"bz{l}"] = _vp(bprev[l] @ np.asarray(w[f"w_inz{l}"], np.float32))
        w[f"bres{l}"] = _vp(bprev[l])
    bL1 = np.asarray(i["m_ln_b"], np.float32)[1]
    # MHA(seq=1) + gate1 collapse: attn_out = h @ (Wo Wv).T + (Wo bv + bo)
    wv = np.asarray(i["attn_in_w"], np.float32)[2 * D_MODEL:]
    bv = np.asarray(i["attn_in_b"], np.float32)[2 * D_MODEL:]
    wo = np.asarray(i["attn_out_w"], np.float32)
    bo = np.asarray(i["attn_out_b"], np.float32)
    wvo = wo @ wv
    bvo = wo @ bv + bo
    g1w = np.asarray(i["gate1_w"], np.float32)
    wg = g1w @ wvo                                                    # [1024, 2048]
    bg = g1w @ bvo + np.asarray(i["gate1_b"], np.float32)
    w["w_gate"] = _t(wg)
    w["b_gate"] = _vp(bg + bL1 @ np.asarray(w["w_gate"], np.float32))
    w["w_g2"] = _t(i["gate2_w"])                                      # [1024, 2]
    w["b_g2"] = np.ascontiguousarray(np.asarray(i["gate2_b"], np.float32).reshape(2, 1))
    w["pm1"] = np.asarray([[1.0], [-1.0]], np.float32)
    w["w_fc"] = _t(i["fc_w"])                                         # [2048, 256]
    w["b_fc"] = _vp(np.asarray(i["fc_b"], np.float32)
                    + bL1 @ np.asarray(w["w_fc"], np.float32)); w["g_fin"] = _vp(i["fin_g"]); w["b_fin"] = _vp(i["fin_b"])
    return w


# ---------------------------------------------------------------- device program

def _emit(ctx, nc, tc, D):
    """Emit the full network under a TileContext. D maps names -> dram APs."""

    def pool(name, bufs, space="SBUF"):
        return ctx.enter_context(tc.tile_pool(name=name, bufs=bufs, space=space))

    consts = pool("consts", 1)
    wt = pool("wt", 4)            # streamed weight tiles
    wres = pool("wres", 1)        # per-layer resident weights
    acts = pool("acts", 2)        # h
    bigp = pool("bigp", 1)        # xp
    medp = pool("medp", 1)        # pre / gh (shared slot)
    inp = pool("inp", 1)          # input feature tiles
    sm = pool("sm", 3)            # small rotating temps (inner-loop)
    szp = pool("szp", 6)          # silu(z) chunks (deferred-consumer depth)
    sm1 = pool("sm1", 1)          # small single-shot temps
    sm2 = pool("sm2", 2)          # output staging
    smc = pool("smc", 4)          # broadcast rows etc.
    psm = pool("psm", 3, space="PSUM")
    psd_p = pool("psd", 2, space="PSUM")
    pss = pool("pss", 2, space="PSUM")
    pst = pool("pst", 1, space="PSUM")

    # constants
    ones_b = consts.tile([P, 1], BF, tag="ones_b")
    nc.vector.memset(ones_b, 1.0)
    ones_f = consts.tile([P, 1], F32, tag="ones_f")
    nc.vector.memset(ones_f, 1.0)
    ident = consts.tile([P, P], F32, tag="ident")
    make_identity(nc, ident)
    eps_t = consts.tile([1, 1], F32, tag="eps")
    nc.vector.memset(eps_t, EPS)

    def cvec(name, cols):
        t = consts.tile([P, cols], F32, tag=name)
        nc.sync.dma_start(t, D[name][:, :])
        return t

    b_img = cvec("b_img", KD); g_img = cvec("g_img", KD); be_img = cvec("be_img", KD)
    b_txt = cvec("b_txt", KD); g_txt = cvec("g_txt", KD); be_txt = cvec("be_txt", KD)
    b_inx = [cvec(f"b_inx{l}", KI) for l in range(2)]
    b_dt = [cvec(f"b_dt{l}", KI) for l in range(2)]
    dvec = [cvec(f"dvec{l}", KI) for l in range(2)]
    g_ln = [cvec(f"g_ln{l}", KM) for l in range(2)]
    b_ln = [cvec(f"b_ln{l}", KM) for l in range(2)]
    bz = [cvec(f"bz{l}", KI) for l in range(2)]
    bres = [cvec(f"bres{l}", KM) for l in range(2)]

    def neg_of(t, cols):
        n = consts.tile([P, cols], F32, tag=f"neg_{t.name}")
        nc.vector.tensor_scalar_mul(n, t, -1.0)
        return n

    ng_img = neg_of(g_img, KD); ng_txt = neg_of(g_txt, KD)
    ng_ln = [neg_of(g_ln[l], KM) for l in range(2)]
    b_gate = cvec("b_gate", KD)
    b_fc = cvec("b_fc", KO); g_fin = cvec("g_fin", KO); b_fin = cvec("b_fin", KO)
    b_g2 = consts.tile([2, 1], F32, tag="b_g2")
    nc.sync.dma_start(b_g2, D["b_g2"][:, :])
    pm1 = consts.tile([2, 1], F32, tag="pm1")
    nc.sync.dma_start(pm1, D["pm1"][:, :])

    # resident small weights
    wg2 = consts.tile([P, KD, 2], BF, tag="wg2")
    nc.sync.dma_start(wg2, D["w_g2"].rearrange("(ko p) n -> p ko n", p=P))
    wfc = consts.tile([P, KM, OUT], BF, tag="wfc")
    nc.sync.dma_start(wfc, D["w_fc"].rearrange("(ko p) n -> p ko n", p=P))

    def linear_prefetch(wname, K, M, mtile=2, ktile=16, groups=1):
        """Pre-issue the weight DMAs for the first `groups` m-groups."""
        wr = D[wname].rearrange("(ko p) n -> p ko n", p=P)
        out = []
        for m0 in range(0, min(M, groups * mtile), mtile):
            mt = min(mtile, M - m0)
            tiles = []
            for k0 in range(0, K, ktile):
                kt = min(ktile, K - k0)
                t = wt.tile([P, kt, mt * P], BF, tag="wt")
                nc.sync.dma_start(t, wr[:, k0:k0 + kt, m0 * P:(m0 + mt) * P])
                tiles.append((k0, kt, t))
            out.append(tiles)
        return out

    def linear(wname, K, M, rhs_fn, out_fn, mtile=2, ktile=16, prefetched=None):
        """out[m] = sum_k W[k,m].T @ rhs[k]; streams weight tiles from DRAM."""
        wr = D[wname].rearrange("(ko p) n -> p ko n", p=P)
        for gi, m0 in enumerate(range(0, M, mtile)):
            mt = min(mtile, M - m0)
            if prefetched is not None and gi < len(prefetched):
                tiles = prefetched[gi]
            else:
                tiles = []
                for k0 in range(0, K, ktile):
                    kt = min(ktile, K - k0)
                    t = wt.tile([P, kt, mt * P], BF, tag="wt")
                    nc.sync.dma_start(t, wr[:, k0:k0 + kt, m0 * P:(m0 + mt) * P])
                    tiles.append((k0, kt, t))
            for m in range(m0, m0 + mt):
                ps = psm.tile([P, BH], F32, tag="mm")
                ki = 0
                for (k0, kt, t) in tiles:
                    for kk in range(kt):
                        nc.tensor.matmul(
                            ps, t[:, kk, (m - m0) * P:(m - m0 + 1) * P], rhs_fn(k0 + kk),
                            start=(ki == 0), stop=(ki == K - 1))
                        ki += 1
                out_fn(m, ps)

    # --- layernorm split: stats accumulate inline with the producing loop ---
    def ln_begin():
        ps_s = pss.tile([1, BH], F32, tag="st")
        ps_q = pss.tile([1, BH], F32, tag="st")
        return (ps_s, ps_q)

    def ln_add(st, chunk, k, K, fp32=False):
        ps_s, ps_q = st
        ones = ones_f if fp32 else ones_b
        nc.tensor.matmul(ps_s, ones, chunk, start=(k == 0), stop=(k == K - 1))
        sq = sm.tile([P, BH], F32 if fp32 else BF, tag="sqk")
        nc.scalar.activation(sq, chunk, AF.Square)
        nc.tensor.matmul(ps_q, ones, sq, start=(k == 0), stop=(k == K - 1))

    def ln_apply(st, src, K, g_t, be_t, dst, d, ng_t=None):
        """rstd via exp(-0.5*ln(var+eps)) so everything stays on one ACT table."""
        ps_s, ps_q = st
        sums = smc.tile([1, BH], F32, tag="r1")
        nc.scalar.copy(sums, ps_s)
        musq = smc.tile([1, BH], F32, tag="r1")
        nc.vector.scalar_tensor_tensor(musq, sums, 1.0 / (d * d), sums,
                                       OP.mult, OP.mult)
        var = smc.tile([1, BH], F32, tag="r1")
        nc.vector.scalar_tensor_tensor(var, ps_q, 1.0 / d, musq,
                                       OP.mult, OP.subtract)
        lv = smc.tile([1, BH], F32, tag="r1")
        nc.scalar.activation(lv, var, AF.Ln, bias=eps_t)
        rstd = smc.tile([1, BH], F32, tag="r1")
        nc.scalar.activation(rstd, lv, AF.Exp, scale=-0.5)
        tmt = smc.tile([1, BH], F32, tag="r1")
        nc.vector.scalar_tensor_tensor(tmt, sums, 1.0 / d, rstd,
                                       OP.mult, OP.mult)
        S = smc.tile([P, BH], F32, tag="bcast")
        nc.gpsimd.partition_broadcast(S, rstd)
        T = smc.tile([P, BH], F32, tag="bcast")
        nc.gpsimd.partition_broadcast(T, tmt)
        if ng_t is not None:
            # beta folded into consumers: dst = x*(S*g) - T*g, two fused passes
            for k in range(K):
                tmp = sm.tile([P, BH], F32, tag="f32tmp")
                nc.vector.scalar_tensor_tensor(tmp, S, g_t[:, k:k + 1],
                                               src[:, k, :], OP.mult, OP.mult)
                nc.vector.scalar_tensor_tensor(dst[:, k, :], T, ng_t[:, k:k + 1],
                                               tmp, OP.mult, OP.add)
        else:
            for k in range(K):
                tmp = sm.tile([P, BH], F32, tag="f32tmp")
                nc.vector.tensor_tensor(tmp, src[:, k, :], S, OP.mult)
                nc.vector.tensor_tensor(tmp, tmp, T, OP.subtract)
                nc.scalar.activation(dst[:, k, :], tmp, AF.Identity,
                                     bias=be_t[:, k:k + 1], scale=g_t[:, k:k + 1])

    for hh in range(NH):
        bsl = slice(hh * BH, (hh + 1) * BH)

        # ---- input projections -> h [P, KM, BH]
        h_t = acts.tile([P, KM, BH], BF, tag="h")
        for (srcn, wname, bvec, gvec, bevec, ngvec, off) in (
            ("imgT", "w_img", b_img, g_img, be_img, ng_img, 0),
            ("txtT", "w_txt", b_txt, g_txt, be_txt, ng_txt, KD),
        ):
            x_in = inp.tile([P, KD, BH], BF, tag="inp")
            nc.sync.dma_start(x_in, D[srcn].rearrange("(ko p) b -> p ko b", p=P)[:, :, bsl])
            prj = medp.tile([P, KD, BH], BF, tag="pre")
            st = ln_begin()

            def prj_out(m, ps, prj=prj, bvec=bvec, st=st):
                nc.scalar.activation(prj[:, m, :], ps, AF.Relu, bias=bvec[:, m:m + 1])
                ln_add(st, prj[:, m, :], m, KD)

            linear(wname, KD, KD,
                   rhs_fn=lambda k, x_in=x_in: x_in[:, k, :],
                   out_fn=prj_out)
            ln_apply(st, prj, KD, gvec, bevec, h_t[:, off:off + KD, :], DIM,
                     ng_t=ngvec)

        # ---- two mamba blocks
        for l in range(2):
            # x' = silu(h @ w_inx + conv_b)  [P, KI, BH]  (native Silu table)
            xp = bigp.tile([P, KI, BH], BF, tag="xp")
            linear(f"w_inx{l}", KM, KI,
                   rhs_fn=lambda k, h_t=h_t: h_t[:, k, :],
                   out_fn=lambda m, ps, xp=xp, l=l: nc.scalar.activation(
                       xp[:, m, :], ps, AF.Silu, bias=b_inx[l][:, m:m + 1]))

            # x_proj m0 -> dblr; dt matmuls for chunks 0..15 interleaved into
            # the m1 (B/C) chain so the ACT softplus (exp/ln) drain never
            # stalls the PE.
            wx = wres.tile([P, KI, 192], BF, tag="wx")
            nc.sync.dma_start(wx, D[f"w_x{l}"].rearrange("(ko p) n -> p ko n", p=P))
            wdt = wres.tile([P, D_INNER], BF, tag="wdt")
            nc.sync.dma_start(wdt, D[f"w_dt{l}"][:, :])
            ps0 = psm.tile([P, BH], F32, tag="mm")
            for k in range(KI):
                nc.tensor.matmul(ps0, wx[:, k, 0:DT_RANK], xp[:, k, :],
                                 start=(k == 0), stop=(k == KI - 1))
            dblr = smc.tile([P, BH], BF, tag="dblr")
            nc.vector.tensor_copy(dblr, ps0)

            def dt_mm(dtg, i, k):
                psd = psd_p.tile([P, BH], F32, tag="psd")
                nc.tensor.matmul(psd, wdt[:, k * P:(k + 1) * P], dblr,
                                 start=True, stop=True)
                ex = sm.tile([P, BH], F32, tag="f32tmp")
                nc.scalar.activation(ex, psd, AF.Exp, bias=b_dt[l][:, k:k + 1])
                nc.scalar.activation(dtg[:, i, :], ex, AF.Ln, bias=ones_f)

            dtg1 = medp.tile([P, KM, BH], BF, tag="pre")
            ps1 = pss.tile([64, BH], F32, tag="st")
            for i in range(KM):
                dt_mm(dtg1, i, i)
                for kk in (2 * i, 2 * i + 1):
                    nc.tensor.matmul(ps1, wx[:, kk, 128:192], xp[:, kk, :],
                                     start=(kk == 0), stop=(kk == KI - 1))
            bcb = sm1.tile([D_STATE, BH], F32, tag="bcb")
            nc.scalar.copy(bcb, ps1[0:D_STATE])
            bcc = sm1.tile([D_STATE, BH], F32, tag="bcc")
            nc.scalar.copy(bcc, ps1[32:32 + D_STATE])
            prod = sm1.tile([D_STATE, BH], F32, tag="prod")
            nc.vector.tensor_tensor(prod, bcb, bcc, OP.mult)
            ps_bc = pss.tile([1, BH], F32, tag="st")
            nc.tensor.matmul(ps_bc, ones_f[:D_STATE], prod, start=True, stop=True)
            bc1 = smc.tile([1, BH], F32, tag="r1")
            nc.scalar.copy(bc1, ps_bc)
            BCt = smc.tile([P, BH], F32, tag="bcast")
            nc.gpsimd.partition_broadcast(BCt, bc1)

            # fused y loop: y = x' * (dt*bc + D) * silu(z), native Silu table.
            wzr = D[f"w_inz{l}"].rearrange("(ko p) n -> p ko n", p=P)

            def z_part(k, kk_in_pair, wz_t):
                psz = psm.tile([P, BH], F32, tag="mm")
                for j in range(KM):
                    nc.tensor.matmul(psz, wz_t[:, j, kk_in_pair * P:(kk_in_pair + 1) * P],
                                     h_t[:, j, :],
                                     start=(j == 0), stop=(j == KM - 1))
                sz = szp.tile([P, BH], BF, tag="sz")
                nc.scalar.activation(sz, psz, AF.Silu, bias=bz[l][:, k:k + 1])
                return sz

            def y_part(k, dtg, i, sz):
                u = sm.tile([P, BH], F32, tag="f32tmp")
                nc.vector.tensor_tensor(u, dtg[:, i, :], BCt, OP.mult)
                nc.vector.scalar_tensor_tensor(u, u, dvec[l][:, k:k + 1],
                                               xp[:, k, :], OP.add, OP.mult)
                nc.vector.tensor_tensor(xp[:, k, :], u, sz, OP.mult)

            def wz_tile(k0):
                t = wt.tile([P, KM, 2 * P], BF, tag="wt")
                nc.sync.dma_start(t, wzr[:, :, k0 * P:(k0 + 2) * P])
                return t

            for k0 in range(0, KM, 2):           # B1: chunks 0..15
                t = wz_tile(k0)
                for kk in range(2):
                    k = k0 + kk
                    y_part(k, dtg1, k, z_part(k, kk, t))
            deferred = []
            for k0 in range(KM, KM + 4, 2):      # B2 head: z only, y deferred
                t = wz_tile(k0)
                for kk in range(2):
                    k = k0 + kk
                    deferred.append((k, z_part(k, kk, t)))
            dtg2 = medp.tile([P, KM, BH], BF, tag="pre")
            for i in range(KM):                  # A2 block
                dt_mm(dtg2, i, KM + i)
            for (k, sz) in deferred:
                y_part(k, dtg2, k - KM, sz)
            wo_pref = linear_prefetch(f"w_out{l}", KI, KM, mtile=1, ktile=KI, groups=2)
            for k0 in range(KM + 4, KI, 2):      # B2 tail
                t = wz_tile(k0)
                for kk in range(2):
                    k = k0 + kk
                    y_part(k, dtg2, k - KM, z_part(k, kk, t))

            # out_proj + residual -> pre (stats inline), then LN -> h
            pre = medp.tile([P, KM, BH], BF, tag="pre")
            st = ln_begin()

            def pre_out(m, ps, pre=pre, h_t=h_t, st=st, l=l):
                nc.vector.scalar_tensor_tensor(pre[:, m, :], h_t[:, m, :],
                                               bres[l][:, m:m + 1], ps,
                                               OP.add, OP.add)
                ln_add(st, pre[:, m, :], m, KM)

            linear(f"w_out{l}", KI, KM,
                   rhs_fn=lambda k, xp=xp: xp[:, k, :],
                   out_fn=pre_out, mtile=1, ktile=KI, prefetched=wo_pref)
            ln_apply(st, pre, KM, g_ln[l], b_ln[l], h_t, D_MODEL, ng_t=ng_ln[l])

        # ---- gate head (folded attn) -> gate weights [BH, 2]
        gh = medp.tile([P, KD, BH], BF, tag="pre")
        linear("w_gate", KM, KD,
               rhs_fn=lambda k, h_t=h_t: h_t[:, k, :],
               out_fn=lambda m, ps, gh=gh: nc.scalar.activation(
                   gh[:, m, :], ps, AF.Relu, bias=b_gate[:, m:m + 1]))
        ps_lg = pss.tile([2, BH], F32, tag="st")
        for k in range(KD):
            nc.tensor.matmul(ps_lg, wg2[:, k, :], gh[:, k, :],
                             start=(k == 0), stop=(k == KD - 1))
        lg = sm1.tile([2, BH], F32, tag="lg")
        nc.scalar.activation(lg, ps_lg, AF.Identity, bias=b_g2)
        ps_d = pss.tile([1, BH], F32, tag="st")
        nc.tensor.matmul(ps_d, pm1, lg, start=True, stop=True)
        # sigmoid via exp: gw0 = 1/(1+exp(-dd))
        egw = smc.tile([1, BH], F32, tag="r1")
        nc.scalar.activation(egw, ps_d, AF.Exp, scale=-1.0)
        nc.vector.tensor_scalar(egw, egw, 1.0, None, OP.add)
        gw0 = smc.tile([1, BH], F32, tag="r1")
        nc.vector.reciprocal(gw0, egw)
        gw1 = smc.tile([1, BH], F32, tag="r1")
        nc.vector.tensor_scalar(gw1, gw0, -1.0, 1.0, OP.mult, OP.add)
        nc.sync.dma_start(D["gw"][bsl, 0:1].rearrange("b t -> t b"), gw0)
        nc.sync.dma_start(D["gw"][bsl, 1:2].rearrange("b t -> t b"), gw1)

        # ---- fc head + final LN -> fused [BH, 256]
        pf = sm1.tile([P, KO, BH], F32, tag="pf")
        st = ln_begin()
        for m in range(KO):
            ps = psm.tile([P, BH], F32, tag="mm")
            for k in range(KM):
                nc.tensor.matmul(ps, wfc[:, k, m * P:(m + 1) * P], h_t[:, k, :],
                                 start=(k == 0), stop=(k == KM - 1))
            nc.scalar.activation(pf[:, m, :], ps, AF.Identity, bias=b_fc[:, m:m + 1])
            ln_add(st, pf[:, m, :], m, KO, fp32=True)
        ff = sm1.tile([P, KO, BH], F32, tag="ff")
        ln_apply(st, pf, KO, g_fin, b_fin, ff, OUT)
        for b4 in range(BH // P):
            ob = sm2.tile([P, OUT], F32, tag="ob")
            for m in range(KO):
                p_t = pst.tile([P, P], F32, tag="tp")
                nc.tensor.transpose(p_t, ff[:, m, b4 * P:(b4 + 1) * P], ident)
                nc.vector.tensor_copy(ob[:, m * P:(m + 1) * P], p_t)
            nc.sync.dma_start(D["fused"][hh * BH + b4 * P: hh * BH + (b4 + 1) * P, :], ob)


# ---------------------------------------------------------------- build + run

_CACHE = {}
_LOCK = threading.Lock()


def _get_program():
    with _LOCK:
        if "nc" in _CACHE:
            return _CACHE["nc"]
        nc = bacc.Bacc("TRN2", target_bir_lowering=False, debug=False,
                       num_devices=N_CORES)
        D = {}

        def din(name, shape, dt):
            D[name] = nc.dram_tensor(name, shape, dt, kind="ExternalInput").ap()

        din("imgT", (DIM, BC), BF)
        din("txtT", (DIM, BC), BF)
        din("w_img", (DIM, DIM), BF)
        din("w_txt", (DIM, DIM), BF)
        for n in ("b_img", "g_img", "be_img", "b_txt", "g_txt", "be_txt", "b_gate"):
            din(n, (P, KD), F32)
        for l in range(2):
            din(f"w_inx{l}", (D_MODEL, D_INNER), BF)
            din(f"w_inz{l}", (D_MODEL, D_INNER), BF)
            din(f"w_x{l}", (D_INNER, 192), BF)
            din(f"w_dt{l}", (DT_RANK, D_INNER), BF)
            din(f"w_out{l}", (D_INNER, D_MODEL), BF)
            for n in (f"b_inx{l}", f"b_dt{l}", f"dvec{l}", f"bz{l}"):
                din(n, (P, KI), F32)
            for n in (f"g_ln{l}", f"b_ln{l}", f"bres{l}"):
                din(n, (P, KM), F32)
        din("w_gate", (D_MODEL, DIM), BF)
        din("w_g2", (DIM, 2), BF)
        din("b_g2", (2, 1), F32)
        din("pm1", (2, 1), F32)
        din("w_fc", (D_MODEL, OUT), BF)
        for n in ("b_fc", "g_fin", "b_fin"):
            din(n, (P, KO), F32)
        D["fused"] = nc.dram_tensor("fused", (BC, OUT), F32, kind="ExternalOutput").ap()
        D["gw"] = nc.dram_tensor("gw", (BC, 2), F32, kind="ExternalOutput").ap()

        from contextlib import ExitStack
        with tile.TileContext(nc) as tc:
            with ExitStack() as es:
                _emit(es, nc, tc, D)
        nc.compile()
        _CACHE["nc"] = nc
        return nc


def _install_ntff_hook():
    """trace=True under axon needs antenv.axon_hooks, absent in this image."""
    if "antenv.axon_hooks" in sys.modules:
        return
    try:
        from trn_agent_boot.trn_boot import _ntff_profile_via_ctypes
        hook = _ntff_profile_via_ctypes("/opt/axon/libaxon_pjrt.so")
    except Exception:
        hook = None
    mod = types.ModuleType("antenv.axon_hooks")
    mod.get_axon_ntff_profile_hook = lambda: hook
    sys.modules["antenv.axon_hooks"] = mod


def kernel(_trace=False, **inputs):
    from concourse.bass_utils import run_bass_kernel_spmd

    _install_ntff_hook()
    nc = _get_program()
    w = _prep_weights(inputs)
    imgT = np.ascontiguousarray(
        np.asarray(inputs["image_features"], np.float32).astype(BF16NP).T)  # [DIM, B]
    txtT = np.ascontiguousarray(
        np.asarray(inputs["text_features"], np.float32).astype(BF16NP).T)
    in_maps = []
    for c in range(N_CORES):
        m = dict(w)
        m["imgT"] = np.ascontiguousarray(imgT[:, c * BC:(c + 1) * BC])
        m["txtT"] = np.ascontiguousarray(txtT[:, c * BC:(c + 1) * BC])
        in_maps.append(m)
    res = run_bass_kernel_spmd(nc, in_maps, core_ids=list(range(N_CORES)),
                               trace=_trace)
    fused = np.concatenate([res.results[c]["fused"] for c in range(N_CORES)], axis=0)
    gw = np.concatenate([res.results[c]["gw"] for c in range(N_CORES)], axis=0)
    if _trace:
        kernel.last_exec_time_ns = res.exec_time_ns
        kernel.last_results = res
    return fused, gw


# revision 28
# speedup vs baseline: 1.0075x; 1.0074x over previous
"""Self-contained Trainium2 Bass kernel for the EnhancedMambaFusion model.

Strategy: pure data-parallel over 8 NeuronCores (batch 8192 -> 1024/core).
On-device everything is computed feature-major ([feature_chunk(128p), batch])
so no activation transposes are needed between layers; weights are
host-transposed to [din, dout] and cast to bf16 (fp32 PSUM accumulation).
Host-side algebraic folding (exact up to fp32 rounding):
  - depthwise-conv last tap scale folded into the in_proj x-half
  - MHA over seq-len 1 is linear: v-proj @ out-proj @ gate1 collapsed into
    a single 2048->1024 matmul feeding the gate softmax
"""

import sys
import threading
import types

sys.path.insert(0, "/opt/trn_rl_repo")

import numpy as np
import ml_dtypes

import concourse.bass as bass
import concourse.bacc as bacc
import concourse.tile as tile
import concourse.mybir as mybir
from concourse.masks import make_identity

# The greedy ACT-table chooser pairs Exp with "exp_and_others" (no Ln) and Ln
# with "natural_log" (no Exp), reloading the LUT between every softplus pair.
# Table *ids* are positional, so keep the list order/length but blank the
# membership of every table except the two this kernel wants; the chooser then
# lands on natural_log_exp_and_others (Exp+Ln) and silu_and_others (Silu) and
# switches only at phase boundaries.
import concourse.hw_specs as _hw_specs
_orig_get_tables = _hw_specs.get_activation_tables

def _patched_tables(arch):
    t = _orig_get_tables(arch)
    keep = {"natural_log_exp_and_others", "silu_and_others"}
    unk = {mybir.ActivationFunctionType.Unknown}
    return {n: (fns if n in keep else unk) for n, fns in t.items()}

bacc.get_activation_tables = _patched_tables

BF16NP = ml_dtypes.bfloat16
BF = mybir.dt.bfloat16
F32 = mybir.dt.float32
AF = mybir.ActivationFunctionType
OP = mybir.AluOpType

N_CORES = 8
B = 8192
DIM = 1024
D_MODEL = 2048
D_INNER = 4096
DT_RANK = 128
D_STATE = 16
OUT = 256
P = 128
BC = B // N_CORES   # batch per core
BH = 512            # half-batch = matmul moving width
NH = BC // BH
EPS = 1e-5

KD = DIM // P        # 8   feature chunks of 1024
KM = D_MODEL // P    # 16  feature chunks of 2048
KI = D_INNER // P    # 32  feature chunks of 4096
KO = OUT // P        # 2   feature chunks of 256


# ---------------------------------------------------------------- host prep

def _t(a):
    """weight [dout, din] fp32 -> bf16 [din, dout] contiguous"""
    return np.ascontiguousarray(np.asarray(a, np.float32).astype(BF16NP).T)


def _vp(v):
    """per-feature vector [d] -> fp32 [128, d/128] (f = ko*128 + p)"""
    v = np.asarray(v, np.float32)
    return np.ascontiguousarray(v.reshape(-1, P).T)


def _prep_weights(i):
    w = {}
    w["w_img"] = _t(i["img_w"])
    w["w_txt"] = _t(i["txt_w"])
    w["b_img"] = _vp(i["img_b"]); w["g_img"] = _vp(i["img_g"]); w["be_img"] = _vp(i["img_beta"])
    w["b_txt"] = _vp(i["txt_b"]); w["g_txt"] = _vp(i["txt_g"]); w["be_txt"] = _vp(i["txt_beta"])
    for l in range(2):
        cw = np.asarray(i["m_conv_w"], np.float32)[l, :, -1]          # [4096] last tap
        win = np.asarray(i["m_in_w"], np.float32)[l]                  # [8192, 2048]
        w[f"w_inx{l}"] = _t(win[:D_INNER] * cw[:, None])              # [2048, 4096]
        w[f"w_inz{l}"] = _t(win[D_INNER:])                            # [2048, 4096]
        w[f"b_inx{l}"] = _vp(np.asarray(i["m_conv_b"], np.float32)[l])
        # x_proj weights: [4096, 192] = [dt_rank(128) | B(16)@128 pad | C(16)@160 pad]
        # so the B/C matmul output lands at 32-aligned partition bases.
        wxT = np.asarray(i["m_x_w"], np.float32)[l].T                 # [4096, 160]
        wx2 = np.zeros((D_INNER, 192), np.float32)
        wx2[:, 0:DT_RANK] = wxT[:, 0:DT_RANK]
        wx2[:, 128:128 + D_STATE] = wxT[:, DT_RANK:DT_RANK + D_STATE]
        wx2[:, 160:160 + D_STATE] = wxT[:, DT_RANK + D_STATE:]
        w[f"w_x{l}"] = np.ascontiguousarray(wx2.astype(BF16NP))
        w[f"w_dt{l}"] = _t(np.asarray(i["m_dt_w"], np.float32)[l])    # [128, 4096]
        w[f"b_dt{l}"] = _vp(np.asarray(i["m_dt_b"], np.float32)[l])
        w[f"dvec{l}"] = _vp(np.asarray(i["m_D"], np.float32)[l])
        w[f"w_out{l}"] = _t(np.asarray(i["m_out_w"], np.float32)[l])  # [4096, 2048]
        w[f"g_ln{l}"] = _vp(np.asarray(i["m_ln_g"], np.float32)[l])
        w[f"b_ln{l}"] = _vp(np.asarray(i["m_ln_b"], np.float32)[l])
    # Fold each h-producing LN's beta into its consumers (h stores the
    # unshifted core); betas: b01 after img/txt, m_ln_b[l] after layer l.
    b01 = np.concatenate([np.asarray(i["img_beta"], np.float32),
                          np.asarray(i["txt_beta"], np.float32)])
    bprev = {0: b01, 1: np.asarray(i["m_ln_g"], np.float32)[0] * 0
             + np.asarray(i["m_ln_b"], np.float32)[0]}
    for l in range(2):
        w[f"b_inx{l}"] = _vp(np.asarray(i["m_conv_b"], np.float32)[l]
                             + bprev[l] @ np.asarray(w[f"w_inx{l}"], np.float32))
        w[f"bz{l}"] = _vp(bprev[l] @ np.asarray(w[f"w_inz{l}"], np.float32))
        w[f"bres{l}"] = _vp(bprev[l])
    bL1 = np.asarray(i["m_ln_b"], np.float32)[1]
    # MHA(seq=1) + gate1 collapse: attn_out = h @ (Wo Wv).T + (Wo bv + bo)
    wv = np.asarray(i["attn_in_w"], np.float32)[2 * D_MODEL:]
    bv = np.asarray(i["attn_in_b"], np.float32)[2 * D_MODEL:]
    wo = np.asarray(i["attn_out_w"], np.float32)
    bo = np.asarray(i["attn_out_b"], np.float32)
    wvo = wo @ wv
    bvo = wo @ bv + bo
    g1w = np.asarray(i["gate1_w"], np.float32)
    wg = g1w @ wvo                                                    # [1024, 2048]
    bg = g1w @ bvo + np.asarray(i["gate1_b"], np.float32)
    w["w_gate"] = _t(wg)
    w["b_gate"] = _vp(bg + bL1 @ np.asarray(w["w_gate"], np.float32))
    w["w_g2"] = _t(i["gate2_w"])                                      # [1024, 2]
    w["b_g2"] = np.ascontiguousarray(np.asarray(i["gate2_b"], np.float32).reshape(2, 1))
    w["pm1"] = np.asarray([[1.0], [-1.0]], np.float32)
    w["w_fc"] = _t(i["fc_w"])                                         # [2048, 256]
    w["b_fc"] = _vp(np.asarray(i["fc_b"], np.float32)
                    + bL1 @ np.asarray(w["w_fc"], np.float32)); w["g_fin"] = _vp(i["fin_g"]); w["b_fin"] = _vp(i["fin_b"])
    return w


# ---------------------------------------------------------------- device program

def _emit(ctx, nc, tc, D):
    """Emit the full network under a TileContext. D maps names -> dram APs."""

    def pool(name, bufs, space="SBUF"):
        return ctx.enter_context(tc.tile_pool(name=name, bufs=bufs, space=space))

    consts = pool("consts", 1)
    wt = pool("wt", 4)            # streamed weight tiles
    wres = pool("wres", 1)        # per-layer resident weights
    acts = pool("acts", 2)        # h
    bigp = pool("bigp", 1)        # xp
    medp = pool("medp", 1)        # pre / gh (shared slot)
    inp = pool("inp", 1)          # input feature tiles
    sm = pool("sm", 3)            # small rotating temps (inner-loop)
    szp = pool("szp", 6)          # silu(z) chunks (deferred-consumer depth)
    sm1 = pool("sm1", 1)          # small single-shot temps
    sm2 = pool("sm2", 2)          # output staging
    smc = pool("smc", 4)          # broadcast rows etc.
    psm = pool("psm", 3, space="PSUM")
    psd_p = pool("psd", 2, space="PSUM")
    pss = pool("pss", 2, space="PSUM")
    pst = pool("pst", 1, space="PSUM")

    # constants
    ones_b = consts.tile([P, 1], BF, tag="ones_b")
    nc.vector.memset(ones_b, 1.0)
    ones_f = consts.tile([P, 1], F32, tag="ones_f")
    nc.vector.memset(ones_f, 1.0)
    ident = consts.tile([P, P], F32, tag="ident")
    make_identity(nc, ident)
    eps_t = consts.tile([1, 1], F32, tag="eps")
    nc.vector.memset(eps_t, EPS)

    def cvec(name, cols):
        t = consts.tile([P, cols], F32, tag=name)
        nc.sync.dma_start(t, D[name][:, :])
        return t

    b_img = cvec("b_img", KD); g_img = cvec("g_img", KD); be_img = cvec("be_img", KD)
    b_txt = cvec("b_txt", KD); g_txt = cvec("g_txt", KD); be_txt = cvec("be_txt", KD)
    b_inx = [cvec(f"b_inx{l}", KI) for l in range(2)]
    b_dt = [cvec(f"b_dt{l}", KI) for l in range(2)]
    dvec = [cvec(f"dvec{l}", KI) for l in range(2)]
    g_ln = [cvec(f"g_ln{l}", KM) for l in range(2)]
    b_ln = [cvec(f"b_ln{l}", KM) for l in range(2)]
    bz = [cvec(f"bz{l}", KI) for l in range(2)]
    bres = [cvec(f"bres{l}", KM) for l in range(2)]

    def neg_of(t, cols):
        n = consts.tile([P, cols], F32, tag=f"neg_{t.name}")
        nc.vector.tensor_scalar_mul(n, t, -1.0)
        return n

    ng_img = neg_of(g_img, KD); ng_txt = neg_of(g_txt, KD)
    ng_ln = [neg_of(g_ln[l], KM) for l in range(2)]
    b_gate = cvec("b_gate", KD)
    b_fc = cvec("b_fc", KO); g_fin = cvec("g_fin", KO); b_fin = cvec("b_fin", KO)
    b_g2 = consts.tile([2, 1], F32, tag="b_g2")
    nc.sync.dma_start(b_g2, D["b_g2"][:, :])
    pm1 = consts.tile([2, 1], F32, tag="pm1")
    nc.sync.dma_start(pm1, D["pm1"][:, :])

    # resident small weights
    wg2 = consts.tile([P, KD, 2], BF, tag="wg2")
    nc.sync.dma_start(wg2, D["w_g2"].rearrange("(ko p) n -> p ko n", p=P))
    wfc = consts.tile([P, KM, OUT], BF, tag="wfc")
    nc.sync.dma_start(wfc, D["w_fc"].rearrange("(ko p) n -> p ko n", p=P))

    def linear_prefetch(wname, K, M, mtile=2, ktile=16, groups=1):
        """Pre-issue the weight DMAs for the first `groups` m-groups."""
        wr = D[wname].rearrange("(ko p) n -> p ko n", p=P)
        out = []
        for m0 in range(0, min(M, groups * mtile), mtile):
            mt = min(mtile, M - m0)
            tiles = []
            for k0 in range(0, K, ktile):
                kt = min(ktile, K - k0)
                t = wt.tile([P, kt, mt * P], BF, tag="wt")
                nc.sync.dma_start(t, wr[:, k0:k0 + kt, m0 * P:(m0 + mt) * P])
                tiles.append((k0, kt, t))
            out.append(tiles)
        return out

    def linear(wname, K, M, rhs_fn, out_fn, mtile=2, ktile=16, prefetched=None):
        """out[m] = sum_k W[k,m].T @ rhs[k]; streams weight tiles from DRAM."""
        wr = D[wname].rearrange("(ko p) n -> p ko n", p=P)
        for gi, m0 in enumerate(range(0, M, mtile)):
            mt = min(mtile, M - m0)
            if prefetched is not None and gi < len(prefetched):
                tiles = prefetched[gi]
            else:
                tiles = []
                for k0 in range(0, K, ktile):
                    kt = min(ktile, K - k0)
                    t = wt.tile([P, kt, mt * P], BF, tag="wt")
                    nc.sync.dma_start(t, wr[:, k0:k0 + kt, m0 * P:(m0 + mt) * P])
                    tiles.append((k0, kt, t))
            for m in range(m0, m0 + mt):
                ps = psm.tile([P, BH], F32, tag="mm")
                ki = 0
                for (k0, kt, t) in tiles:
                    for kk in range(kt):
                        nc.tensor.matmul(
                            ps, t[:, kk, (m - m0) * P:(m - m0 + 1) * P], rhs_fn(k0 + kk),
                            start=(ki == 0), stop=(ki == K - 1))
                        ki += 1
                out_fn(m, ps)

    # --- layernorm split: stats accumulate inline with the producing loop ---
    def ln_begin():
        ps_s = pss.tile([1, BH], F32, tag="st")
        ps_q = pss.tile([1, BH], F32, tag="st")
        return (ps_s, ps_q)

    def ln_add(st, chunk, k, K, fp32=False):
        ps_s, ps_q = st
        ones = ones_f if fp32 else ones_b
        nc.tensor.matmul(ps_s, ones, chunk, start=(k == 0), stop=(k == K - 1))
        sq = sm.tile([P, BH], F32 if fp32 else BF, tag="sqk")
        nc.scalar.activation(sq, chunk, AF.Square)
        nc.tensor.matmul(ps_q, ones, sq, start=(k == 0), stop=(k == K - 1))

    def ln_apply(st, src, K, g_t, be_t, dst, d, ng_t=None):
        """rstd via exp(-0.5*ln(var+eps)) so everything stays on one ACT table."""
        ps_s, ps_q = st
        musq = smc.tile([1, BH], F32, tag="r1")
        nc.scalar.activation(musq, ps_s, AF.Square, scale=1.0 / d)
        var = smc.tile([1, BH], F32, tag="r1")
        nc.vector.scalar_tensor_tensor(var, ps_q, 1.0 / d, musq,
                                       OP.mult, OP.subtract)
        lv = smc.tile([1, BH], F32, tag="r1")
        nc.scalar.activation(lv, var, AF.Ln, bias=eps_t)
        rstd = smc.tile([1, BH], F32, tag="r1")
        nc.scalar.activation(rstd, lv, AF.Exp, scale=-0.5)
        tmt = smc.tile([1, BH], F32, tag="r1")
        nc.vector.scalar_tensor_tensor(tmt, ps_s, 1.0 / d, rstd,
                                       OP.mult, OP.mult)
        S = smc.tile([P, BH], F32, tag="bcast")
        nc.gpsimd.partition_broadcast(S, rstd)
        T = smc.tile([P, BH], F32, tag="bcast")
        nc.gpsimd.partition_broadcast(T, tmt)
        if ng_t is not None:
            # beta folded into consumers: dst = x*(S*g) - T*g, two fused passes
            for k in range(K):
                tmp = sm.tile([P, BH], F32, tag="f32tmp")
                nc.vector.scalar_tensor_tensor(tmp, S, g_t[:, k:k + 1],
                                               src[:, k, :], OP.mult, OP.mult)
                nc.vector.scalar_tensor_tensor(dst[:, k, :], T, ng_t[:, k:k + 1],
                                               tmp, OP.mult, OP.add)
        else:
            for k in range(K):
                tmp = sm.tile([P, BH], F32, tag="f32tmp")
                nc.vector.tensor_tensor(tmp, src[:, k, :], S, OP.mult)
                nc.vector.tensor_tensor(tmp, tmp, T, OP.subtract)
                nc.scalar.activation(dst[:, k, :], tmp, AF.Identity,
                                     bias=be_t[:, k:k + 1], scale=g_t[:, k:k + 1])

    for hh in range(NH):
        bsl = slice(hh * BH, (hh + 1) * BH)

        # ---- input projections -> h [P, KM, BH]
        h_t = acts.tile([P, KM, BH], BF, tag="h")
        for (srcn, wname, bvec, gvec, bevec, ngvec, off) in (
            ("imgT", "w_img", b_img, g_img, be_img, ng_img, 0),
            ("txtT", "w_txt", b_txt, g_txt, be_txt, ng_txt, KD),
        ):
            x_in = inp.tile([P, KD, BH], BF, tag="inp")
            nc.sync.dma_start(x_in, D[srcn].rearrange("(ko p) b -> p ko b", p=P)[:, :, bsl])
            prj = medp.tile([P, KD, BH], BF, tag="pre")
            st = ln_begin()

            def prj_out(m, ps, prj=prj, bvec=bvec, st=st):
                nc.scalar.activation(prj[:, m, :], ps, AF.Relu, bias=bvec[:, m:m + 1])
                ln_add(st, prj[:, m, :], m, KD)

            linear(wname, KD, KD,
                   rhs_fn=lambda k, x_in=x_in: x_in[:, k, :],
                   out_fn=prj_out)
            ln_apply(st, prj, KD, gvec, bevec, h_t[:, off:off + KD, :], DIM,
                     ng_t=ngvec)

        # ---- two mamba blocks
        for l in range(2):
            # x' = silu(h @ w_inx + conv_b)  [P, KI, BH]  (native Silu table)
            xp = bigp.tile([P, KI, BH], BF, tag="xp")
            linear(f"w_inx{l}", KM, KI,
                   rhs_fn=lambda k, h_t=h_t: h_t[:, k, :],
                   out_fn=lambda m, ps, xp=xp, l=l: nc.scalar.activation(
                       xp[:, m, :], ps, AF.Silu, bias=b_inx[l][:, m:m + 1]))

            # x_proj m0 -> dblr; dt matmuls for chunks 0..15 interleaved into
            # the m1 (B/C) chain so the ACT softplus (exp/ln) drain never
            # stalls the PE.
            wx = wres.tile([P, KI, 192], BF, tag="wx")
            nc.sync.dma_start(wx, D[f"w_x{l}"].rearrange("(ko p) n -> p ko n", p=P))
            wdt = wres.tile([P, D_INNER], BF, tag="wdt")
            nc.sync.dma_start(wdt, D[f"w_dt{l}"][:, :])
            ps0 = psm.tile([P, BH], F32, tag="mm")
            for k in range(KI):
                nc.tensor.matmul(ps0, wx[:, k, 0:DT_RANK], xp[:, k, :],
                                 start=(k == 0), stop=(k == KI - 1))
            dblr = smc.tile([P, BH], BF, tag="dblr")
            nc.vector.tensor_copy(dblr, ps0)

            def dt_mm(dtg, i, k, wide=False):
                if wide and i % 2:
                    psd = pss.tile([P, BH], F32, tag="st")
                else:
                    psd = psd_p.tile([P, BH], F32, tag="psd")
                nc.tensor.matmul(psd, wdt[:, k * P:(k + 1) * P], dblr,
                                 start=True, stop=True)
                ex = sm.tile([P, BH], F32, tag="f32tmp")
                nc.scalar.activation(ex, psd, AF.Exp, bias=b_dt[l][:, k:k + 1])
                nc.scalar.activation(dtg[:, i, :], ex, AF.Ln, bias=ones_f)

            dtg1 = medp.tile([P, KM, BH], BF, tag="pre")
            ps1 = pss.tile([64, BH], F32, tag="st")
            for i in range(KM):
                dt_mm(dtg1, i, i)
                for kk in (2 * i, 2 * i + 1):
                    nc.tensor.matmul(ps1, wx[:, kk, 128:192], xp[:, kk, :],
                                     start=(kk == 0), stop=(kk == KI - 1))
            bcb = sm1.tile([D_STATE, BH], F32, tag="bcb")
            nc.scalar.copy(bcb, ps1[0:D_STATE])
            bcc = sm1.tile([D_STATE, BH], F32, tag="bcc")
            nc.scalar.copy(bcc, ps1[32:32 + D_STATE])
            prod = sm1.tile([D_STATE, BH], F32, tag="prod")
            nc.vector.tensor_tensor(prod, bcb, bcc, OP.mult)
            ps_bc = pss.tile([1, BH], F32, tag="st")
            nc.tensor.matmul(ps_bc, ones_f[:D_STATE], prod, start=True, stop=True)
            bc1 = smc.tile([1, BH], F32, tag="r1")
            nc.scalar.copy(bc1, ps_bc)
            BCt = smc.tile([P, BH], F32, tag="bcast")
            nc.gpsimd.partition_broadcast(BCt, bc1)

            # fused y loop: y = x' * (dt*bc + D) * silu(z), native Silu table.
            wzr = D[f"w_inz{l}"].rearrange("(ko p) n -> p ko n", p=P)

            def z_part(k, kk_in_pair, wz_t):
                psz = psm.tile([P, BH], F32, tag="mm")
                for j in range(KM):
                    nc.tensor.matmul(psz, wz_t[:, j, kk_in_pair * P:(kk_in_pair + 1) * P],
                                     h_t[:, j, :],
                                     start=(j == 0), stop=(j == KM - 1))
                sz = szp.tile([P, BH], BF, tag="sz")
                nc.scalar.activation(sz, psz, AF.Silu, bias=bz[l][:, k:k + 1])
                return sz

            def y_part(k, dtg, i, sz):
                u = sm.tile([P, BH], F32, tag="f32tmp")
                nc.vector.tensor_tensor(u, dtg[:, i, :], BCt, OP.mult)
                nc.vector.scalar_tensor_tensor(u, u, dvec[l][:, k:k + 1],
                                               xp[:, k, :], OP.add, OP.mult)
                nc.vector.tensor_tensor(xp[:, k, :], u, sz, OP.mult)

            def wz_tile(k0):
                t = wt.tile([P, KM, 2 * P], BF, tag="wt")
                nc.sync.dma_start(t, wzr[:, :, k0 * P:(k0 + 2) * P])
                return t

            for k0 in range(0, KM, 2):           # B1: chunks 0..15
                t = wz_tile(k0)
                for kk in range(2):
                    k = k0 + kk
                    y_part(k, dtg1, k, z_part(k, kk, t))
            deferred = []
            for k0 in range(KM, KM + 4, 2):      # B2 head: z only, y deferred
                t = wz_tile(k0)
                for kk in range(2):
                    k = k0 + kk
                    deferred.append((k, z_part(k, kk, t)))
            dtg2 = medp.tile([P, KM, BH], BF, tag="pre")
            for i in range(KM):                  # A2 block
                dt_mm(dtg2, i, KM + i, wide=True)
            for (k, sz) in deferred:
                y_part(k, dtg2, k - KM, sz)
            wo_pref = linear_prefetch(f"w_out{l}", KI, KM, mtile=1, ktile=KI, groups=2)
            for k0 in range(KM + 4, KI, 2):      # B2 tail
                t = wz_tile(k0)
                for kk in range(2):
                    k = k0 + kk
                    y_part(k, dtg2, k - KM, z_part(k, kk, t))

            # out_proj + residual -> pre (stats inline), then LN -> h
            pre = medp.tile([P, KM, BH], BF, tag="pre")
            st = ln_begin()

            def pre_out(m, ps, pre=pre, h_t=h_t, st=st, l=l):
                nc.vector.scalar_tensor_tensor(pre[:, m, :], h_t[:, m, :],
                                               bres[l][:, m:m + 1], ps,
                                               OP.add, OP.add)
                ln_add(st, pre[:, m, :], m, KM)

            linear(f"w_out{l}", KI, KM,
                   rhs_fn=lambda k, xp=xp: xp[:, k, :],
                   out_fn=pre_out, mtile=1, ktile=KI, prefetched=wo_pref)
            ln_apply(st, pre, KM, g_ln[l], b_ln[l], h_t, D_MODEL, ng_t=ng_ln[l])

        # ---- gate head (folded attn) -> gate weights [BH, 2]
        gh = medp.tile([P, KD, BH], BF, tag="pre")
        linear("w_gate", KM, KD,
               rhs_fn=lambda k, h_t=h_t: h_t[:, k, :],
               out_fn=lambda m, ps, gh=gh: nc.scalar.activation(
                   gh[:, m, :], ps, AF.Relu, bias=b_gate[:, m:m + 1]))
        ps_lg = pss.tile([2, BH], F32, tag="st")
        for k in range(KD):
            nc.tensor.matmul(ps_lg, wg2[:, k, :], gh[:, k, :],
                             start=(k == 0), stop=(k == KD - 1))
        lg = sm1.tile([2, BH], F32, tag="lg")
        nc.scalar.activation(lg, ps_lg, AF.Identity, bias=b_g2)
        ps_d = pss.tile([1, BH], F32, tag="st")
        nc.tensor.matmul(ps_d, pm1, lg, start=True, stop=True)
        # sigmoid via exp: gw0 = 1/(1+exp(-dd))
        egw = smc.tile([1, BH], F32, tag="r1")
        nc.scalar.activation(egw, ps_d, AF.Exp, scale=-1.0)
        nc.vector.tensor_scalar(egw, egw, 1.0, None, OP.add)
        gw0 = smc.tile([1, BH], F32, tag="r1")
        nc.vector.reciprocal(gw0, egw)
        gw1 = smc.tile([1, BH], F32, tag="r1")
        nc.vector.tensor_scalar(gw1, gw0, -1.0, 1.0, OP.mult, OP.add)
        nc.sync.dma_start(D["gw"][bsl, 0:1].rearrange("b t -> t b"), gw0)
        nc.sync.dma_start(D["gw"][bsl, 1:2].rearrange("b t -> t b"), gw1)

        # ---- fc head + final LN -> fused [BH, 256]
        pf = sm1.tile([P, KO, BH], F32, tag="pf")
        st = ln_begin()
        for m in range(KO):
            ps = psm.tile([P, BH], F32, tag="mm")
            for k in range(KM):
                nc.tensor.matmul(ps, wfc[:, k, m * P:(m + 1) * P], h_t[:, k, :],
                                 start=(k == 0), stop=(k == KM - 1))
            nc.scalar.activation(pf[:, m, :], ps, AF.Identity, bias=b_fc[:, m:m + 1])
            ln_add(st, pf[:, m, :], m, KO, fp32=True)
        ff = sm1.tile([P, KO, BH], F32, tag="ff")
        ln_apply(st, pf, KO, g_fin, b_fin, ff, OUT)
        for b4 in range(BH // P):
            ob = sm2.tile([P, OUT], F32, tag="ob")
            for m in range(KO):
                p_t = pst.tile([P, P], F32, tag="tp")
                nc.tensor.transpose(p_t, ff[:, m, b4 * P:(b4 + 1) * P], ident)
                nc.vector.tensor_copy(ob[:, m * P:(m + 1) * P], p_t)
            nc.sync.dma_start(D["fused"][hh * BH + b4 * P: hh * BH + (b4 + 1) * P, :], ob)


# ---------------------------------------------------------------- build + run

_CACHE = {}
_LOCK = threading.Lock()


def _get_program():
    with _LOCK:
        if "nc" in _CACHE:
            return _CACHE["nc"]
        nc = bacc.Bacc("TRN2", target_bir_lowering=False, debug=False,
                       num_devices=N_CORES)
        D = {}

        def din(name, shape, dt):
            D[name] = nc.dram_tensor(name, shape, dt, kind="ExternalInput").ap()

        din("imgT", (DIM, BC), BF)
        din("txtT", (DIM, BC), BF)
        din("w_img", (DIM, DIM), BF)
        din("w_txt", (DIM, DIM), BF)
        for n in ("b_img", "g_img", "be_img", "b_txt", "g_txt", "be_txt", "b_gate"):
            din(n, (P, KD), F32)
        for l in range(2):
            din(f"w_inx{l}", (D_MODEL, D_INNER), BF)
            din(f"w_inz{l}", (D_MODEL, D_INNER), BF)
            din(f"w_x{l}", (D_INNER, 192), BF)
            din(f"w_dt{l}", (DT_RANK, D_INNER), BF)
            din(f"w_out{l}", (D_INNER, D_MODEL), BF)
            for n in (f"b_inx{l}", f"b_dt{l}", f"dvec{l}", f"bz{l}"):
                din(n, (P, KI), F32)
            for n in (f"g_ln{l}", f"b_ln{l}", f"bres{l}"):
                din(n, (P, KM), F32)
        din("w_gate", (D_MODEL, DIM), BF)
        din("w_g2", (DIM, 2), BF)
        din("b_g2", (2, 1), F32)
        din("pm1", (2, 1), F32)
        din("w_fc", (D_MODEL, OUT), BF)
        for n in ("b_fc", "g_fin", "b_fin"):
            din(n, (P, KO), F32)
        D["fused"] = nc.dram_tensor("fused", (BC, OUT), F32, kind="ExternalOutput").ap()
        D["gw"] = nc.dram_tensor("gw", (BC, 2), F32, kind="ExternalOutput").ap()

        from contextlib import ExitStack
        with tile.TileContext(nc) as tc:
            with ExitStack() as es:
                _emit(es, nc, tc, D)
        nc.compile()
        _CACHE["nc"] = nc
        return nc


def _install_ntff_hook():
    """trace=True under axon needs antenv.axon_hooks, absent in this image."""
    if "antenv.axon_hooks" in sys.modules:
        return
    try:
        from trn_agent_boot.trn_boot import _ntff_profile_via_ctypes
        hook = _ntff_profile_via_ctypes("/opt/axon/libaxon_pjrt.so")
    except Exception:
        hook = None
    mod = types.ModuleType("antenv.axon_hooks")
    mod.get_axon_ntff_profile_hook = lambda: hook
    sys.modules["antenv.axon_hooks"] = mod


def kernel(_trace=False, **inputs):
    from concourse.bass_utils import run_bass_kernel_spmd

    _install_ntff_hook()
    nc = _get_program()
    w = _prep_weights(inputs)
    imgT = np.ascontiguousarray(
        np.asarray(inputs["image_features"], np.float32).astype(BF16NP).T)  # [DIM, B]
    txtT = np.ascontiguousarray(
        np.asarray(inputs["text_features"], np.float32).astype(BF16NP).T)
    in_maps = []
    for c in range(N_CORES):
        m = dict(w)
        m["imgT"] = np.ascontiguousarray(imgT[:, c * BC:(c + 1) * BC])
        m["txtT"] = np.ascontiguousarray(txtT[:, c * BC:(c + 1) * BC])
        in_maps.append(m)
    res = run_bass_kernel_spmd(nc, in_maps, core_ids=list(range(N_CORES)),
                               trace=_trace)
    fused = np.concatenate([res.results[c]["fused"] for c in range(N_CORES)], axis=0)
    gw = np.concatenate([res.results[c]["gw"] for c in range(N_CORES)], axis=0)
    if _trace:
        kernel.last_exec_time_ns = res.exec_time_ns
        kernel.last_results = res
    return fused, gw


# revision 29
# speedup vs baseline: 1.0327x; 1.0250x over previous
"""Self-contained Trainium2 Bass kernel for the EnhancedMambaFusion model.

Strategy: pure data-parallel over 8 NeuronCores (batch 8192 -> 1024/core).
On-device everything is computed feature-major ([feature_chunk(128p), batch])
so no activation transposes are needed between layers; weights are
host-transposed to [din, dout] and cast to bf16 (fp32 PSUM accumulation).
Host-side algebraic folding (exact up to fp32 rounding):
  - depthwise-conv last tap scale folded into the in_proj x-half
  - MHA over seq-len 1 is linear: v-proj @ out-proj @ gate1 collapsed into
    a single 2048->1024 matmul feeding the gate softmax
"""

import sys
import threading
import types

sys.path.insert(0, "/opt/trn_rl_repo")

import numpy as np
import ml_dtypes

import concourse.bass as bass
import concourse.bacc as bacc
import concourse.tile as tile
import concourse.mybir as mybir
from concourse.masks import make_identity

# The greedy ACT-table chooser pairs Exp with "exp_and_others" (no Ln) and Ln
# with "natural_log" (no Exp), reloading the LUT between every softplus pair.
# Table *ids* are positional, so keep the list order/length but blank the
# membership of every table except the two this kernel wants; the chooser then
# lands on natural_log_exp_and_others (Exp+Ln) and silu_and_others (Silu) and
# switches only at phase boundaries.
import concourse.hw_specs as _hw_specs
_orig_get_tables = _hw_specs.get_activation_tables

def _patched_tables(arch):
    t = _orig_get_tables(arch)
    keep = {"natural_log_exp_and_others", "silu_and_others"}
    unk = {mybir.ActivationFunctionType.Unknown}
    return {n: (fns if n in keep else unk) for n, fns in t.items()}

bacc.get_activation_tables = _patched_tables

BF16NP = ml_dtypes.bfloat16
BF = mybir.dt.bfloat16
F32 = mybir.dt.float32
AF = mybir.ActivationFunctionType
OP = mybir.AluOpType

N_CORES = 8
B = 8192
DIM = 1024
D_MODEL = 2048
D_INNER = 4096
DT_RANK = 128
D_STATE = 16
OUT = 256
P = 128
BC = B // N_CORES   # batch per core
BH = 512            # half-batch = matmul moving width
NH = BC // BH
EPS = 1e-5

KD = DIM // P        # 8   feature chunks of 1024
KM = D_MODEL // P    # 16  feature chunks of 2048
KI = D_INNER // P    # 32  feature chunks of 4096
KO = OUT // P        # 2   feature chunks of 256


# ---------------------------------------------------------------- host prep

def _t(a):
    """weight [dout, din] fp32 -> bf16 [din, dout] contiguous"""
    return np.ascontiguousarray(np.asarray(a, np.float32).astype(BF16NP).T)


def _vp(v):
    """per-feature vector [d] -> fp32 [128, d/128] (f = ko*128 + p)"""
    v = np.asarray(v, np.float32)
    return np.ascontiguousarray(v.reshape(-1, P).T)


def _prep_weights(i):
    w = {}
    w["w_img"] = _t(i["img_w"])
    w["w_txt"] = _t(i["txt_w"])
    w["b_img"] = _vp(i["img_b"]); w["g_img"] = _vp(i["img_g"]); w["be_img"] = _vp(i["img_beta"])
    w["b_txt"] = _vp(i["txt_b"]); w["g_txt"] = _vp(i["txt_g"]); w["be_txt"] = _vp(i["txt_beta"])
    for l in range(2):
        cw = np.asarray(i["m_conv_w"], np.float32)[l, :, -1]          # [4096] last tap
        win = np.asarray(i["m_in_w"], np.float32)[l]                  # [8192, 2048]
        w[f"w_inx{l}"] = _t(win[:D_INNER] * cw[:, None])              # [2048, 4096]
        w[f"w_inz{l}"] = _t(win[D_INNER:])                            # [2048, 4096]
        w[f"b_inx{l}"] = _vp(np.asarray(i["m_conv_b"], np.float32)[l])
        # x_proj weights: [4096, 192] = [dt_rank(128) | B(16)@128 pad | C(16)@160 pad]
        # so the B/C matmul output lands at 32-aligned partition bases.
        wxT = np.asarray(i["m_x_w"], np.float32)[l].T                 # [4096, 160]
        wx2 = np.zeros((D_INNER, 192), np.float32)
        wx2[:, 0:DT_RANK] = wxT[:, 0:DT_RANK]
        wx2[:, 128:128 + D_STATE] = wxT[:, DT_RANK:DT_RANK + D_STATE]
        wx2[:, 160:160 + D_STATE] = wxT[:, DT_RANK + D_STATE:]
        w[f"w_x{l}"] = np.ascontiguousarray(wx2.astype(BF16NP))
        w[f"w_dt{l}"] = _t(np.asarray(i["m_dt_w"], np.float32)[l])    # [128, 4096]
        w[f"b_dt{l}"] = _vp(np.asarray(i["m_dt_b"], np.float32)[l])
        w[f"dvec{l}"] = _vp(np.asarray(i["m_D"], np.float32)[l])
        w[f"w_out{l}"] = _t(np.asarray(i["m_out_w"], np.float32)[l])  # [4096, 2048]
        w[f"g_ln{l}"] = _vp(np.asarray(i["m_ln_g"], np.float32)[l])
        w[f"b_ln{l}"] = _vp(np.asarray(i["m_ln_b"], np.float32)[l])
    # Fold each h-producing LN's beta into its consumers (h stores the
    # unshifted core); betas: b01 after img/txt, m_ln_b[l] after layer l.
    b01 = np.concatenate([np.asarray(i["img_beta"], np.float32),
                          np.asarray(i["txt_beta"], np.float32)])
    bprev = {0: b01, 1: np.asarray(i["m_ln_g"], np.float32)[0] * 0
             + np.asarray(i["m_ln_b"], np.float32)[0]}
    for l in range(2):
        w[f"b_inx{l}"] = _vp(np.asarray(i["m_conv_b"], np.float32)[l]
                             + bprev[l] @ np.asarray(w[f"w_inx{l}"], np.float32))
        w[f"bz{l}"] = _vp(bprev[l] @ np.asarray(w[f"w_inz{l}"], np.float32))
        w[f"bres{l}"] = _vp(bprev[l])
    bL1 = np.asarray(i["m_ln_b"], np.float32)[1]
    # MHA(seq=1) + gate1 collapse: attn_out = h @ (Wo Wv).T + (Wo bv + bo)
    wv = np.asarray(i["attn_in_w"], np.float32)[2 * D_MODEL:]
    bv = np.asarray(i["attn_in_b"], np.float32)[2 * D_MODEL:]
    wo = np.asarray(i["attn_out_w"], np.float32)
    bo = np.asarray(i["attn_out_b"], np.float32)
    wvo = wo @ wv
    bvo = wo @ bv + bo
    g1w = np.asarray(i["gate1_w"], np.float32)
    wg = g1w @ wvo                                                    # [1024, 2048]
    bg = g1w @ bvo + np.asarray(i["gate1_b"], np.float32)
    w["w_gate"] = _t(wg)
    w["b_gate"] = _vp(bg + bL1 @ np.asarray(w["w_gate"], np.float32))
    w["w_g2"] = _t(i["gate2_w"])                                      # [1024, 2]
    w["b_g2"] = np.ascontiguousarray(np.asarray(i["gate2_b"], np.float32).reshape(2, 1))
    w["pm1"] = np.asarray([[1.0], [-1.0]], np.float32)
    w["w_fc"] = _t(i["fc_w"])                                         # [2048, 256]
    w["b_fc"] = _vp(np.asarray(i["fc_b"], np.float32)
                    + bL1 @ np.asarray(w["w_fc"], np.float32)); w["g_fin"] = _vp(i["fin_g"]); w["b_fin"] = _vp(i["fin_b"])
    return w


# ---------------------------------------------------------------- device program

def _emit(ctx, nc, tc, D):
    """Emit the full network under a TileContext. D maps names -> dram APs."""

    def pool(name, bufs, space="SBUF"):
        return ctx.enter_context(tc.tile_pool(name=name, bufs=bufs, space=space))

    consts = pool("consts", 1)
    wt = pool("wt", 4)            # streamed weight tiles
    wres = pool("wres", 1)        # per-layer resident weights
    acts = pool("acts", 2)        # h
    bigp = pool("bigp", 1)        # xp
    medp = pool("medp", 1)        # pre / gh (shared slot)
    inp = pool("inp", 1)          # input feature tiles
    sm = pool("sm", 3)            # small rotating temps (inner-loop)
    szp = pool("szp", 6)          # silu(z) chunks (deferred-consumer depth)
    sm1 = pool("sm1", 1)          # small single-shot temps
    sm2 = pool("sm2", 2)          # output staging
    smc = pool("smc", 4)          # broadcast rows etc.
    psm = pool("psm", 3, space="PSUM")
    psd_p = pool("psd", 2, space="PSUM")
    pss = pool("pss", 2, space="PSUM")
    pst = pool("pst", 1, space="PSUM")

    # constants
    ones_b = consts.tile([P, 1], BF, tag="ones_b")
    nc.vector.memset(ones_b, 1.0)
    ones_f = consts.tile([P, 1], F32, tag="ones_f")
    nc.vector.memset(ones_f, 1.0)
    ident = consts.tile([P, P], F32, tag="ident")
    make_identity(nc, ident)
    eps_t = consts.tile([1, 1], F32, tag="eps")
    nc.vector.memset(eps_t, EPS)

    def cvec(name, cols):
        t = consts.tile([P, cols], F32, tag=name)
        nc.sync.dma_start(t, D[name][:, :])
        return t

    b_img = cvec("b_img", KD); g_img = cvec("g_img", KD); be_img = cvec("be_img", KD)
    b_txt = cvec("b_txt", KD); g_txt = cvec("g_txt", KD); be_txt = cvec("be_txt", KD)
    b_inx = [cvec(f"b_inx{l}", KI) for l in range(2)]
    b_dt = [cvec(f"b_dt{l}", KI) for l in range(2)]
    dvec = [cvec(f"dvec{l}", KI) for l in range(2)]
    g_ln = [cvec(f"g_ln{l}", KM) for l in range(2)]
    b_ln = [cvec(f"b_ln{l}", KM) for l in range(2)]
    bz = [cvec(f"bz{l}", KI) for l in range(2)]
    bres = [cvec(f"bres{l}", KM) for l in range(2)]

    def neg_of(t, cols):
        n = consts.tile([P, cols], F32, tag=f"neg_{t.name}")
        nc.vector.tensor_scalar_mul(n, t, -1.0)
        return n

    ng_img = neg_of(g_img, KD); ng_txt = neg_of(g_txt, KD)
    ng_ln = [neg_of(g_ln[l], KM) for l in range(2)]
    b_gate = cvec("b_gate", KD)
    b_fc = cvec("b_fc", KO); g_fin = cvec("g_fin", KO); b_fin = cvec("b_fin", KO)
    b_g2 = consts.tile([2, 1], F32, tag="b_g2")
    nc.sync.dma_start(b_g2, D["b_g2"][:, :])
    pm1 = consts.tile([2, 1], F32, tag="pm1")
    nc.sync.dma_start(pm1, D["pm1"][:, :])

    # resident small weights
    wg2 = consts.tile([P, KD, 2], BF, tag="wg2")
    nc.sync.dma_start(wg2, D["w_g2"].rearrange("(ko p) n -> p ko n", p=P))
    wfc = consts.tile([P, KM, OUT], BF, tag="wfc")
    nc.sync.dma_start(wfc, D["w_fc"].rearrange("(ko p) n -> p ko n", p=P))

    def linear_prefetch(wname, K, M, mtile=2, ktile=16, groups=1):
        """Pre-issue the weight DMAs for the first `groups` m-groups."""
        wr = D[wname].rearrange("(ko p) n -> p ko n", p=P)
        out = []
        for m0 in range(0, min(M, groups * mtile), mtile):
            mt = min(mtile, M - m0)
            tiles = []
            for k0 in range(0, K, ktile):
                kt = min(ktile, K - k0)
                t = wt.tile([P, kt, mt * P], BF, tag="wt")
                nc.sync.dma_start(t, wr[:, k0:k0 + kt, m0 * P:(m0 + mt) * P])
                tiles.append((k0, kt, t))
            out.append(tiles)
        return out

    def linear(wname, K, M, rhs_fn, out_fn, mtile=2, ktile=16, prefetched=None):
        """out[m] = sum_k W[k,m].T @ rhs[k]; streams weight tiles from DRAM."""
        wr = D[wname].rearrange("(ko p) n -> p ko n", p=P)
        for gi, m0 in enumerate(range(0, M, mtile)):
            mt = min(mtile, M - m0)
            if prefetched is not None and gi < len(prefetched):
                tiles = prefetched[gi]
            else:
                tiles = []
                for k0 in range(0, K, ktile):
                    kt = min(ktile, K - k0)
                    t = wt.tile([P, kt, mt * P], BF, tag="wt")
                    nc.sync.dma_start(t, wr[:, k0:k0 + kt, m0 * P:(m0 + mt) * P])
                    tiles.append((k0, kt, t))
            for m in range(m0, m0 + mt):
                ps = psm.tile([P, BH], F32, tag="mm")
                ki = 0
                for (k0, kt, t) in tiles:
                    for kk in range(kt):
                        nc.tensor.matmul(
                            ps, t[:, kk, (m - m0) * P:(m - m0 + 1) * P], rhs_fn(k0 + kk),
                            start=(ki == 0), stop=(ki == K - 1))
                        ki += 1
                out_fn(m, ps)

    # --- layernorm split: stats accumulate inline with the producing loop ---
    def ln_begin():
        ps_s = pss.tile([1, BH], F32, tag="st")
        ps_q = pss.tile([1, BH], F32, tag="st")
        return (ps_s, ps_q)

    def ln_add(st, chunk, k, K, fp32=False):
        ps_s, ps_q = st
        ones = ones_f if fp32 else ones_b
        nc.tensor.matmul(ps_s, ones, chunk, start=(k == 0), stop=(k == K - 1))
        sq = sm.tile([P, BH], F32 if fp32 else BF, tag="sqk")
        nc.scalar.activation(sq, chunk, AF.Square)
        nc.tensor.matmul(ps_q, ones, sq, start=(k == 0), stop=(k == K - 1))

    def ln_apply(st, src, K, g_t, be_t, dst, d, ng_t=None):
        """rstd via exp(-0.5*ln(var+eps)) so everything stays on one ACT table."""
        ps_s, ps_q = st
        musq = smc.tile([1, BH], F32, tag="r1")
        nc.scalar.activation(musq, ps_s, AF.Square, scale=1.0 / d)
        var = smc.tile([1, BH], F32, tag="r1")
        nc.vector.scalar_tensor_tensor(var, ps_q, 1.0 / d, musq,
                                       OP.mult, OP.subtract)
        lv = smc.tile([1, BH], F32, tag="r1")
        nc.scalar.activation(lv, var, AF.Ln, bias=eps_t)
        rstd = smc.tile([1, BH], F32, tag="r1")
        nc.scalar.activation(rstd, lv, AF.Exp, scale=-0.5)
        tmt = smc.tile([1, BH], F32, tag="r1")
        nc.vector.scalar_tensor_tensor(tmt, ps_s, 1.0 / d, rstd,
                                       OP.mult, OP.mult)
        S = smc.tile([P, BH], F32, tag="bcast")
        nc.gpsimd.partition_broadcast(S, rstd)
        T = smc.tile([P, BH], F32, tag="bcast")
        nc.gpsimd.partition_broadcast(T, tmt)
        if ng_t is not None:
            # beta folded into consumers: dst = x*(S*g) - T*g, two fused passes
            for k in range(K):
                tmp = sm.tile([P, BH], F32, tag="f32tmp")
                nc.vector.scalar_tensor_tensor(tmp, S, g_t[:, k:k + 1],
                                               src[:, k, :], OP.mult, OP.mult)
                nc.vector.scalar_tensor_tensor(dst[:, k, :], T, ng_t[:, k:k + 1],
                                               tmp, OP.mult, OP.add)
        else:
            for k in range(K):
                tmp = sm.tile([P, BH], F32, tag="f32tmp")
                nc.vector.tensor_tensor(tmp, src[:, k, :], S, OP.mult)
                nc.vector.tensor_tensor(tmp, tmp, T, OP.subtract)
                nc.scalar.activation(dst[:, k, :], tmp, AF.Identity,
                                     bias=be_t[:, k:k + 1], scale=g_t[:, k:k + 1])

    for hh in range(NH):
        bsl = slice(hh * BH, (hh + 1) * BH)

        # ---- input projections -> h [P, KM, BH]
        h_t = acts.tile([P, KM, BH], BF, tag="h")
        for (srcn, wname, bvec, gvec, bevec, ngvec, off) in (
            ("imgT", "w_img", b_img, g_img, be_img, ng_img, 0),
            ("txtT", "w_txt", b_txt, g_txt, be_txt, ng_txt, KD),
        ):
            x_in = inp.tile([P, KD, BH], BF, tag="inp")
            nc.sync.dma_start(x_in, D[srcn].rearrange("(ko p) b -> p ko b", p=P)[:, :, bsl])
            prj = medp.tile([P, KD, BH], BF, tag="pre")
            st = ln_begin()

            def prj_out(m, ps, prj=prj, bvec=bvec, st=st):
                nc.scalar.activation(prj[:, m, :], ps, AF.Relu, bias=bvec[:, m:m + 1])
                ln_add(st, prj[:, m, :], m, KD)

            linear(wname, KD, KD,
                   rhs_fn=lambda k, x_in=x_in: x_in[:, k, :],
                   out_fn=prj_out)
            ln_apply(st, prj, KD, gvec, bevec, h_t[:, off:off + KD, :], DIM,
                     ng_t=ngvec)

        # ---- two mamba blocks
        for l in range(2):
            # x' = silu(h @ w_inx + conv_b)  [P, KI, BH]  (native Silu table)
            xp = bigp.tile([P, KI, BH], BF, tag="xp")
            linear(f"w_inx{l}", KM, KI,
                   rhs_fn=lambda k, h_t=h_t: h_t[:, k, :],
                   out_fn=lambda m, ps, xp=xp, l=l: nc.scalar.activation(
                       xp[:, m, :], ps, AF.Silu, bias=b_inx[l][:, m:m + 1]))

            # x_proj m0 -> dblr; dt matmuls for chunks 0..15 interleaved into
            # the m1 (B/C) chain so the ACT softplus (exp/ln) drain never
            # stalls the PE.
            wx = wres.tile([P, KI, 192], BF, tag="wx")
            nc.sync.dma_start(wx, D[f"w_x{l}"].rearrange("(ko p) n -> p ko n", p=P))
            wdt = wres.tile([P, D_INNER], BF, tag="wdt")
            nc.sync.dma_start(wdt, D[f"w_dt{l}"][:, :])
            ps0 = psm.tile([P, BH], F32, tag="mm")
            for k in range(KI):
                nc.tensor.matmul(ps0, wx[:, k, 0:DT_RANK], xp[:, k, :],
                                 start=(k == 0), stop=(k == KI - 1))
            dblr = smc.tile([P, BH], BF, tag="dblr")
            nc.vector.tensor_copy(dblr, ps0)

            def dt_mm(dtg, i, k, wide=False):
                if wide and i % 2:
                    psd = pss.tile([P, BH], F32, tag="st")
                else:
                    psd = psd_p.tile([P, BH], F32, tag="psd")
                nc.tensor.matmul(psd, wdt[:, k * P:(k + 1) * P], dblr,
                                 start=True, stop=True)
                ex = sm.tile([P, BH], F32, tag="f32tmp")
                nc.scalar.activation(ex, psd, AF.Exp, bias=b_dt[l][:, k:k + 1])
                nc.scalar.activation(dtg[:, i, :], ex, AF.Ln, bias=ones_f)

            dtg1 = medp.tile([P, KM, BH], BF, tag="pre")
            ps1 = pss.tile([64, BH], F32, tag="st")
            for i in range(KM):
                dt_mm(dtg1, i, i)
                for kk in (2 * i, 2 * i + 1):
                    nc.tensor.matmul(ps1, wx[:, kk, 128:192], xp[:, kk, :],
                                     start=(kk == 0), stop=(kk == KI - 1))
            bcb = sm1.tile([D_STATE, BH], F32, tag="bcb")
            nc.scalar.copy(bcb, ps1[0:D_STATE])
            bcc = sm1.tile([D_STATE, BH], F32, tag="bcc")
            nc.scalar.copy(bcc, ps1[32:32 + D_STATE])
            prod = sm1.tile([D_STATE, BH], F32, tag="prod")
            nc.vector.tensor_tensor(prod, bcb, bcc, OP.mult)
            ps_bc = pss.tile([1, BH], F32, tag="st")
            nc.tensor.matmul(ps_bc, ones_f[:D_STATE], prod, start=True, stop=True)
            bc1 = smc.tile([1, BH], F32, tag="r1")
            nc.scalar.copy(bc1, ps_bc)
            BCt = smc.tile([P, BH], F32, tag="bcast")
            nc.gpsimd.partition_broadcast(BCt, bc1)

            # fused y loop: y = x' * (dt*bc + D) * silu(z), native Silu table.
            wzr = D[f"w_inz{l}"].rearrange("(ko p) n -> p ko n", p=P)

            def z_part(k, kk_in_pair, wz_t):
                psz = psm.tile([P, BH], F32, tag="mm")
                for j in range(KM):
                    nc.tensor.matmul(psz, wz_t[:, j, kk_in_pair * P:(kk_in_pair + 1) * P],
                                     h_t[:, j, :],
                                     start=(j == 0), stop=(j == KM - 1))
                sz = szp.tile([P, BH], BF, tag="sz")
                nc.scalar.activation(sz, psz, AF.Silu, bias=bz[l][:, k:k + 1])
                return sz

            def y_part(k, dtg, i, sz):
                u = sm.tile([P, BH], F32, tag="f32tmp")
                nc.vector.tensor_tensor(u, dtg[:, i, :], BCt, OP.mult)
                nc.vector.scalar_tensor_tensor(u, u, dvec[l][:, k:k + 1],
                                               xp[:, k, :], OP.add, OP.mult)
                nc.vector.tensor_tensor(xp[:, k, :], u, sz, OP.mult)

            def wz_tile(k0):
                t = wt.tile([P, KM, 2 * P], BF, tag="wt")
                nc.sync.dma_start(t, wzr[:, :, k0 * P:(k0 + 2) * P])
                return t

            for k0 in range(0, KM, 2):           # B1: chunks 0..15
                t = wz_tile(k0)
                for kk in range(2):
                    k = k0 + kk
                    y_part(k, dtg1, k, z_part(k, kk, t))
            # B2: dt for chunks 16..31 computed in 4-MM micro-blocks two
            # chunks ahead of use; 4 rotating banks (psd + idle stats slots)
            # mean the PE never waits on the ACT softplus drain.
            dtg2 = medp.tile([P, KM, BH], BF, tag="pre")
            wo_pref = None
            for k0 in range(KM, KI, 2):
                j = (k0 - KM) // 2
                if j % 2 == 0 and j < 8:
                    base = KM + 2 * j
                    for i in range(4):
                        dt_mm(dtg2, base - KM + i, base + i, wide=True)
                if k0 == KI - 4:
                    wo_pref = linear_prefetch(f"w_out{l}", KI, KM,
                                              mtile=1, ktile=KI, groups=2)
                t = wz_tile(k0)
                for kk in range(2):
                    k = k0 + kk
                    y_part(k, dtg2, k - KM, z_part(k, kk, t))

            # out_proj + residual -> pre (stats inline), then LN -> h
            pre = medp.tile([P, KM, BH], BF, tag="pre")
            st = ln_begin()

            def pre_out(m, ps, pre=pre, h_t=h_t, st=st, l=l):
                nc.vector.scalar_tensor_tensor(pre[:, m, :], h_t[:, m, :],
                                               bres[l][:, m:m + 1], ps,
                                               OP.add, OP.add)
                ln_add(st, pre[:, m, :], m, KM)

            linear(f"w_out{l}", KI, KM,
                   rhs_fn=lambda k, xp=xp: xp[:, k, :],
                   out_fn=pre_out, mtile=1, ktile=KI, prefetched=wo_pref)
            ln_apply(st, pre, KM, g_ln[l], b_ln[l], h_t, D_MODEL, ng_t=ng_ln[l])

        # ---- gate head (folded attn) -> gate weights [BH, 2]
        gh = medp.tile([P, KD, BH], BF, tag="pre")
        linear("w_gate", KM, KD,
               rhs_fn=lambda k, h_t=h_t: h_t[:, k, :],
               out_fn=lambda m, ps, gh=gh: nc.scalar.activation(
                   gh[:, m, :], ps, AF.Relu, bias=b_gate[:, m:m + 1]))
        ps_lg = pss.tile([2, BH], F32, tag="st")
        for k in range(KD):
            nc.tensor.matmul(ps_lg, wg2[:, k, :], gh[:, k, :],
                             start=(k == 0), stop=(k == KD - 1))
        lg = sm1.tile([2, BH], F32, tag="lg")
        nc.scalar.activation(lg, ps_lg, AF.Identity, bias=b_g2)
        ps_d = pss.tile([1, BH], F32, tag="st")
        nc.tensor.matmul(ps_d, pm1, lg, start=True, stop=True)
        # sigmoid via exp: gw0 = 1/(1+exp(-dd))
        egw = smc.tile([1, BH], F32, tag="r1")
        nc.scalar.activation(egw, ps_d, AF.Exp, scale=-1.0)
        nc.vector.tensor_scalar(egw, egw, 1.0, None, OP.add)
        gw0 = smc.tile([1, BH], F32, tag="r1")
        nc.vector.reciprocal(gw0, egw)
        gw1 = smc.tile([1, BH], F32, tag="r1")
        nc.vector.tensor_scalar(gw1, gw0, -1.0, 1.0, OP.mult, OP.add)
        nc.sync.dma_start(D["gw"][bsl, 0:1].rearrange("b t -> t b"), gw0)
        nc.sync.dma_start(D["gw"][bsl, 1:2].rearrange("b t -> t b"), gw1)

        # ---- fc head + final LN -> fused [BH, 256]
        pf = sm1.tile([P, KO, BH], F32, tag="pf")
        st = ln_begin()
        for m in range(KO):
            ps = psm.tile([P, BH], F32, tag="mm")
            for k in range(KM):
                nc.tensor.matmul(ps, wfc[:, k, m * P:(m + 1) * P], h_t[:, k, :],
                                 start=(k == 0), stop=(k == KM - 1))
            nc.scalar.activation(pf[:, m, :], ps, AF.Identity, bias=b_fc[:, m:m + 1])
            ln_add(st, pf[:, m, :], m, KO, fp32=True)
        ff = sm1.tile([P, KO, BH], F32, tag="ff")
        ln_apply(st, pf, KO, g_fin, b_fin, ff, OUT)
        for b4 in range(BH // P):
            ob = sm2.tile([P, OUT], F32, tag="ob")
            for m in range(KO):
                p_t = pst.tile([P, P], F32, tag="tp")
                nc.tensor.transpose(p_t, ff[:, m, b4 * P:(b4 + 1) * P], ident)
                nc.vector.tensor_copy(ob[:, m * P:(m + 1) * P], p_t)
            nc.sync.dma_start(D["fused"][hh * BH + b4 * P: hh * BH + (b4 + 1) * P, :], ob)


# ---------------------------------------------------------------- build + run

_CACHE = {}
_LOCK = threading.Lock()


def _get_program():
    with _LOCK:
        if "nc" in _CACHE:
            return _CACHE["nc"]
        nc = bacc.Bacc("TRN2", target_bir_lowering=False, debug=False,
                       num_devices=N_CORES)
        D = {}

        def din(name, shape, dt):
            D[name] = nc.dram_tensor(name, shape, dt, kind="ExternalInput").ap()

        din("imgT", (DIM, BC), BF)
        din("txtT", (DIM, BC), BF)
        din("w_img", (DIM, DIM), BF)
        din("w_txt", (DIM, DIM), BF)
        for n in ("b_img", "g_img", "be_img", "b_txt", "g_txt", "be_txt", "b_gate"):
            din(n, (P, KD), F32)
        for l in range(2):
            din(f"w_inx{l}", (D_MODEL, D_INNER), BF)
            din(f"w_inz{l}", (D_MODEL, D_INNER), BF)
            din(f"w_x{l}", (D_INNER, 192), BF)
            din(f"w_dt{l}", (DT_RANK, D_INNER), BF)
            din(f"w_out{l}", (D_INNER, D_MODEL), BF)
            for n in (f"b_inx{l}", f"b_dt{l}", f"dvec{l}", f"bz{l}"):
                din(n, (P, KI), F32)
            for n in (f"g_ln{l}", f"b_ln{l}", f"bres{l}"):
                din(n, (P, KM), F32)
        din("w_gate", (D_MODEL, DIM), BF)
        din("w_g2", (DIM, 2), BF)
        din("b_g2", (2, 1), F32)
        din("pm1", (2, 1), F32)
        din("w_fc", (D_MODEL, OUT), BF)
        for n in ("b_fc", "g_fin", "b_fin"):
            din(n, (P, KO), F32)
        D["fused"] = nc.dram_tensor("fused", (BC, OUT), F32, kind="ExternalOutput").ap()
        D["gw"] = nc.dram_tensor("gw", (BC, 2), F32, kind="ExternalOutput").ap()

        from contextlib import ExitStack
        with tile.TileContext(nc) as tc:
            with ExitStack() as es:
                _emit(es, nc, tc, D)
        nc.compile()
        _CACHE["nc"] = nc
        return nc


def _install_ntff_hook():
    """trace=True under axon needs antenv.axon_hooks, absent in this image."""
    if "antenv.axon_hooks" in sys.modules:
        return
    try:
        from trn_agent_boot.trn_boot import _ntff_profile_via_ctypes
        hook = _ntff_profile_via_ctypes("/opt/axon/libaxon_pjrt.so")
    except Exception:
        hook = None
    mod = types.ModuleType("antenv.axon_hooks")
    mod.get_axon_ntff_profile_hook = lambda: hook
    sys.modules["antenv.axon_hooks"] = mod


def kernel(_trace=False, **inputs):
    from concourse.bass_utils import run_bass_kernel_spmd

    _install_ntff_hook()
    nc = _get_program()
    w = _prep_weights(inputs)
    imgT = np.ascontiguousarray(
        np.asarray(inputs["image_features"], np.float32).astype(BF16NP).T)  # [DIM, B]
    txtT = np.ascontiguousarray(
        np.asarray(inputs["text_features"], np.float32).astype(BF16NP).T)
    in_maps = []
    for c in range(N_CORES):
        m = dict(w)
        m["imgT"] = np.ascontiguousarray(imgT[:, c * BC:(c + 1) * BC])
        m["txtT"] = np.ascontiguousarray(txtT[:, c * BC:(c + 1) * BC])
        in_maps.append(m)
    res = run_bass_kernel_spmd(nc, in_maps, core_ids=list(range(N_CORES)),
                               trace=_trace)
    fused = np.concatenate([res.results[c]["fused"] for c in range(N_CORES)], axis=0)
    gw = np.concatenate([res.results[c]["gw"] for c in range(N_CORES)], axis=0)
    if _trace:
        kernel.last_exec_time_ns = res.exec_time_ns
        kernel.last_results = res
    return fused, gw


# revision 30
# speedup vs baseline: 1.0349x; 1.0021x over previous
"""Self-contained Trainium2 Bass kernel for the EnhancedMambaFusion model.

Strategy: pure data-parallel over 8 NeuronCores (batch 8192 -> 1024/core).
On-device everything is computed feature-major ([feature_chunk(128p), batch])
so no activation transposes are needed between layers; weights are
host-transposed to [din, dout] and cast to bf16 (fp32 PSUM accumulation).
Host-side algebraic folding (exact up to fp32 rounding):
  - depthwise-conv last tap scale folded into the in_proj x-half
  - MHA over seq-len 1 is linear: v-proj @ out-proj @ gate1 collapsed into
    a single 2048->1024 matmul feeding the gate softmax
"""

import sys
import threading
import types

sys.path.insert(0, "/opt/trn_rl_repo")

import numpy as np
import ml_dtypes

import concourse.bass as bass
import concourse.bacc as bacc
import concourse.tile as tile
import concourse.mybir as mybir
from concourse.masks import make_identity

# The greedy ACT-table chooser pairs Exp with "exp_and_others" (no Ln) and Ln
# with "natural_log" (no Exp), reloading the LUT between every softplus pair.
# Table *ids* are positional, so keep the list order/length but blank the
# membership of every table except the two this kernel wants; the chooser then
# lands on natural_log_exp_and_others (Exp+Ln) and silu_and_others (Silu) and
# switches only at phase boundaries.
import concourse.hw_specs as _hw_specs
_orig_get_tables = _hw_specs.get_activation_tables

def _patched_tables(arch):
    t = _orig_get_tables(arch)
    keep = {"natural_log_exp_and_others", "silu_and_others"}
    unk = {mybir.ActivationFunctionType.Unknown}
    return {n: (fns if n in keep else unk) for n, fns in t.items()}

bacc.get_activation_tables = _patched_tables

BF16NP = ml_dtypes.bfloat16
BF = mybir.dt.bfloat16
F32 = mybir.dt.float32
AF = mybir.ActivationFunctionType
OP = mybir.AluOpType

N_CORES = 8
B = 8192
DIM = 1024
D_MODEL = 2048
D_INNER = 4096
DT_RANK = 128
D_STATE = 16
OUT = 256
P = 128
BC = B // N_CORES   # batch per core
BH = 512            # half-batch = matmul moving width
NH = BC // BH
EPS = 1e-5

KD = DIM // P        # 8   feature chunks of 1024
KM = D_MODEL // P    # 16  feature chunks of 2048
KI = D_INNER // P    # 32  feature chunks of 4096
KO = OUT // P        # 2   feature chunks of 256


# ---------------------------------------------------------------- host prep

def _t(a):
    """weight [dout, din] fp32 -> bf16 [din, dout] contiguous"""
    return np.ascontiguousarray(np.asarray(a, np.float32).astype(BF16NP).T)


def _vp(v):
    """per-feature vector [d] -> fp32 [128, d/128] (f = ko*128 + p)"""
    v = np.asarray(v, np.float32)
    return np.ascontiguousarray(v.reshape(-1, P).T)


def _prep_weights(i):
    w = {}
    w["w_img"] = _t(i["img_w"])
    w["w_txt"] = _t(i["txt_w"])
    w["b_img"] = _vp(i["img_b"]); w["g_img"] = _vp(i["img_g"]); w["be_img"] = _vp(i["img_beta"])
    w["b_txt"] = _vp(i["txt_b"]); w["g_txt"] = _vp(i["txt_g"]); w["be_txt"] = _vp(i["txt_beta"])
    for l in range(2):
        cw = np.asarray(i["m_conv_w"], np.float32)[l, :, -1]          # [4096] last tap
        win = np.asarray(i["m_in_w"], np.float32)[l]                  # [8192, 2048]
        w[f"w_inx{l}"] = _t(win[:D_INNER] * cw[:, None])              # [2048, 4096]
        w[f"w_inz{l}"] = _t(win[D_INNER:])                            # [2048, 4096]
        w[f"b_inx{l}"] = _vp(np.asarray(i["m_conv_b"], np.float32)[l])
        # x_proj weights: [4096, 192] = [dt_rank(128) | B(16)@128 pad | C(16)@160 pad]
        # so the B/C matmul output lands at 32-aligned partition bases.
        wxT = np.asarray(i["m_x_w"], np.float32)[l].T                 # [4096, 160]
        wx2 = np.zeros((D_INNER, 192), np.float32)
        wx2[:, 0:DT_RANK] = wxT[:, 0:DT_RANK]
        wx2[:, 128:128 + D_STATE] = wxT[:, DT_RANK:DT_RANK + D_STATE]
        wx2[:, 160:160 + D_STATE] = wxT[:, DT_RANK + D_STATE:]
        w[f"w_x{l}"] = np.ascontiguousarray(wx2.astype(BF16NP))
        w[f"w_dt{l}"] = _t(np.asarray(i["m_dt_w"], np.float32)[l])    # [128, 4096]
        w[f"b_dt{l}"] = _vp(np.asarray(i["m_dt_b"], np.float32)[l])
        w[f"dvec{l}"] = _vp(np.asarray(i["m_D"], np.float32)[l])
        w[f"w_out{l}"] = _t(np.asarray(i["m_out_w"], np.float32)[l])  # [4096, 2048]
        w[f"g_ln{l}"] = _vp(np.asarray(i["m_ln_g"], np.float32)[l])
        w[f"b_ln{l}"] = _vp(np.asarray(i["m_ln_b"], np.float32)[l])
    # Fold each h-producing LN's beta into its consumers (h stores the
    # unshifted core); betas: b01 after img/txt, m_ln_b[l] after layer l.
    b01 = np.concatenate([np.asarray(i["img_beta"], np.float32),
                          np.asarray(i["txt_beta"], np.float32)])
    bprev = {0: b01, 1: np.asarray(i["m_ln_g"], np.float32)[0] * 0
             + np.asarray(i["m_ln_b"], np.float32)[0]}
    for l in range(2):
        w[f"b_inx{l}"] = _vp(np.asarray(i["m_conv_b"], np.float32)[l]
                             + bprev[l] @ np.asarray(w[f"w_inx{l}"], np.float32))
        w[f"bz{l}"] = _vp(bprev[l] @ np.asarray(w[f"w_inz{l}"], np.float32))
        w[f"bres{l}"] = _vp(bprev[l])
    bL1 = np.asarray(i["m_ln_b"], np.float32)[1]
    # MHA(seq=1) + gate1 collapse: attn_out = h @ (Wo Wv).T + (Wo bv + bo)
    wv = np.asarray(i["attn_in_w"], np.float32)[2 * D_MODEL:]
    bv = np.asarray(i["attn_in_b"], np.float32)[2 * D_MODEL:]
    wo = np.asarray(i["attn_out_w"], np.float32)
    bo = np.asarray(i["attn_out_b"], np.float32)
    wvo = wo @ wv
    bvo = wo @ bv + bo
    g1w = np.asarray(i["gate1_w"], np.float32)
    wg = g1w @ wvo                                                    # [1024, 2048]
    bg = g1w @ bvo + np.asarray(i["gate1_b"], np.float32)
    w["w_gate"] = _t(wg)
    w["b_gate"] = _vp(bg + bL1 @ np.asarray(w["w_gate"], np.float32))
    w["w_g2"] = _t(i["gate2_w"])                                      # [1024, 2]
    w["b_g2"] = np.ascontiguousarray(np.asarray(i["gate2_b"], np.float32).reshape(2, 1))
    w["pm1"] = np.asarray([[1.0], [-1.0]], np.float32)
    w["w_fc"] = _t(i["fc_w"])                                         # [2048, 256]
    w["b_fc"] = _vp(np.asarray(i["fc_b"], np.float32)
                    + bL1 @ np.asarray(w["w_fc"], np.float32)); w["g_fin"] = _vp(i["fin_g"]); w["b_fin"] = _vp(i["fin_b"])
    return w


# ---------------------------------------------------------------- device program

def _emit(ctx, nc, tc, D):
    """Emit the full network under a TileContext. D maps names -> dram APs."""

    def pool(name, bufs, space="SBUF"):
        return ctx.enter_context(tc.tile_pool(name=name, bufs=bufs, space=space))

    consts = pool("consts", 1)
    wt = pool("wt", 4)            # streamed weight tiles
    wres = pool("wres", 1)        # per-layer resident weights
    acts = pool("acts", 2)        # h
    bigp = pool("bigp", 1)        # xp
    medp = pool("medp", 1)        # pre / gh (shared slot)
    inp = pool("inp", 1)          # input feature tiles
    sm = pool("sm", 3)            # small rotating temps (inner-loop)
    szp = pool("szp", 6)          # silu(z) chunks (deferred-consumer depth)
    sm1 = pool("sm1", 1)          # small single-shot temps
    sm2 = pool("sm2", 2)          # output staging
    smc = pool("smc", 4)          # broadcast rows etc.
    psm = pool("psm", 3, space="PSUM")
    psd_p = pool("psd", 2, space="PSUM")
    pss = pool("pss", 2, space="PSUM")
    pst = pool("pst", 1, space="PSUM")

    # constants
    ones_b = consts.tile([P, 1], BF, tag="ones_b")
    nc.vector.memset(ones_b, 1.0)
    ones_f = consts.tile([P, 1], F32, tag="ones_f")
    nc.vector.memset(ones_f, 1.0)
    ident = consts.tile([P, P], F32, tag="ident")
    make_identity(nc, ident)
    eps_t = consts.tile([1, 1], F32, tag="eps")
    nc.vector.memset(eps_t, EPS)

    def cvec(name, cols):
        t = consts.tile([P, cols], F32, tag=name)
        nc.sync.dma_start(t, D[name][:, :])
        return t

    b_img = cvec("b_img", KD); g_img = cvec("g_img", KD); be_img = cvec("be_img", KD)
    b_txt = cvec("b_txt", KD); g_txt = cvec("g_txt", KD); be_txt = cvec("be_txt", KD)
    b_inx = [cvec(f"b_inx{l}", KI) for l in range(2)]
    b_dt = [cvec(f"b_dt{l}", KI) for l in range(2)]
    dvec = [cvec(f"dvec{l}", KI) for l in range(2)]
    g_ln = [cvec(f"g_ln{l}", KM) for l in range(2)]
    b_ln = [cvec(f"b_ln{l}", KM) for l in range(2)]
    bz = [cvec(f"bz{l}", KI) for l in range(2)]
    bres = [cvec(f"bres{l}", KM) for l in range(2)]

    def neg_of(t, cols):
        n = consts.tile([P, cols], F32, tag=f"neg_{t.name}")
        nc.vector.tensor_scalar_mul(n, t, -1.0)
        return n

    ng_img = neg_of(g_img, KD); ng_txt = neg_of(g_txt, KD)
    ng_ln = [neg_of(g_ln[l], KM) for l in range(2)]
    b_gate = cvec("b_gate", KD)
    b_fc = cvec("b_fc", KO); g_fin = cvec("g_fin", KO); b_fin = cvec("b_fin", KO)
    b_g2 = consts.tile([2, 1], F32, tag="b_g2")
    nc.sync.dma_start(b_g2, D["b_g2"][:, :])
    pm1 = consts.tile([2, 1], F32, tag="pm1")
    nc.sync.dma_start(pm1, D["pm1"][:, :])

    # resident small weights (DMAs deferred into the first half's mamba phase)
    wg2 = consts.tile([P, KD, 2], BF, tag="wg2")
    wfc = consts.tile([P, KM, OUT], BF, tag="wfc")

    def linear_prefetch(wname, K, M, mtile=2, ktile=16, groups=1):
        """Pre-issue the weight DMAs for the first `groups` m-groups."""
        wr = D[wname].rearrange("(ko p) n -> p ko n", p=P)
        out = []
        for m0 in range(0, min(M, groups * mtile), mtile):
            mt = min(mtile, M - m0)
            tiles = []
            for k0 in range(0, K, ktile):
                kt = min(ktile, K - k0)
                t = wt.tile([P, kt, mt * P], BF, tag="wt")
                nc.sync.dma_start(t, wr[:, k0:k0 + kt, m0 * P:(m0 + mt) * P])
                tiles.append((k0, kt, t))
            out.append(tiles)
        return out

    def linear(wname, K, M, rhs_fn, out_fn, mtile=2, ktile=16, prefetched=None):
        """out[m] = sum_k W[k,m].T @ rhs[k]; streams weight tiles from DRAM."""
        wr = D[wname].rearrange("(ko p) n -> p ko n", p=P)
        for gi, m0 in enumerate(range(0, M, mtile)):
            mt = min(mtile, M - m0)
            if prefetched is not None and gi < len(prefetched):
                tiles = prefetched[gi]
            else:
                tiles = []
                for k0 in range(0, K, ktile):
                    kt = min(ktile, K - k0)
                    t = wt.tile([P, kt, mt * P], BF, tag="wt")
                    nc.sync.dma_start(t, wr[:, k0:k0 + kt, m0 * P:(m0 + mt) * P])
                    tiles.append((k0, kt, t))
            for m in range(m0, m0 + mt):
                ps = psm.tile([P, BH], F32, tag="mm")
                ki = 0
                for (k0, kt, t) in tiles:
                    for kk in range(kt):
                        nc.tensor.matmul(
                            ps, t[:, kk, (m - m0) * P:(m - m0 + 1) * P], rhs_fn(k0 + kk),
                            start=(ki == 0), stop=(ki == K - 1))
                        ki += 1
                out_fn(m, ps)

    # --- layernorm split: stats accumulate inline with the producing loop ---
    def ln_begin():
        ps_s = pss.tile([1, BH], F32, tag="st")
        ps_q = pss.tile([1, BH], F32, tag="st")
        return (ps_s, ps_q)

    def ln_add(st, chunk, k, K, fp32=False):
        ps_s, ps_q = st
        ones = ones_f if fp32 else ones_b
        nc.tensor.matmul(ps_s, ones, chunk, start=(k == 0), stop=(k == K - 1))
        sq = sm.tile([P, BH], F32 if fp32 else BF, tag="sqk")
        nc.scalar.activation(sq, chunk, AF.Square)
        nc.tensor.matmul(ps_q, ones, sq, start=(k == 0), stop=(k == K - 1))

    def ln_apply(st, src, K, g_t, be_t, dst, d, ng_t=None):
        """rstd via exp(-0.5*ln(var+eps)) so everything stays on one ACT table."""
        ps_s, ps_q = st
        musq = smc.tile([1, BH], F32, tag="r1")
        nc.scalar.activation(musq, ps_s, AF.Square, scale=1.0 / d)
        var = smc.tile([1, BH], F32, tag="r1")
        nc.vector.scalar_tensor_tensor(var, ps_q, 1.0 / d, musq,
                                       OP.mult, OP.subtract)
        lv = smc.tile([1, BH], F32, tag="r1")
        nc.scalar.activation(lv, var, AF.Ln, bias=eps_t)
        rstd = smc.tile([1, BH], F32, tag="r1")
        nc.scalar.activation(rstd, lv, AF.Exp, scale=-0.5)
        tmt = smc.tile([1, BH], F32, tag="r1")
        nc.vector.scalar_tensor_tensor(tmt, ps_s, 1.0 / d, rstd,
                                       OP.mult, OP.mult)
        S = smc.tile([P, BH], F32, tag="bcast")
        nc.gpsimd.partition_broadcast(S, rstd)
        T = smc.tile([P, BH], F32, tag="bcast")
        nc.gpsimd.partition_broadcast(T, tmt)
        if ng_t is not None:
            # beta folded into consumers: dst = x*(S*g) - T*g, two fused passes
            for k in range(K):
                tmp = sm.tile([P, BH], F32, tag="f32tmp")
                nc.vector.scalar_tensor_tensor(tmp, S, g_t[:, k:k + 1],
                                               src[:, k, :], OP.mult, OP.mult)
                nc.vector.scalar_tensor_tensor(dst[:, k, :], T, ng_t[:, k:k + 1],
                                               tmp, OP.mult, OP.add)
        else:
            for k in range(K):
                tmp = sm.tile([P, BH], F32, tag="f32tmp")
                nc.vector.tensor_tensor(tmp, src[:, k, :], S, OP.mult)
                nc.vector.tensor_tensor(tmp, tmp, T, OP.subtract)
                nc.scalar.activation(dst[:, k, :], tmp, AF.Identity,
                                     bias=be_t[:, k:k + 1], scale=g_t[:, k:k + 1])

    for hh in range(NH):
        bsl = slice(hh * BH, (hh + 1) * BH)

        # ---- input projections -> h [P, KM, BH]
        h_t = acts.tile([P, KM, BH], BF, tag="h")
        for (srcn, wname, bvec, gvec, bevec, ngvec, off) in (
            ("imgT", "w_img", b_img, g_img, be_img, ng_img, 0),
            ("txtT", "w_txt", b_txt, g_txt, be_txt, ng_txt, KD),
        ):
            x_in = inp.tile([P, KD, BH], BF, tag="inp")
            nc.sync.dma_start(x_in, D[srcn].rearrange("(ko p) b -> p ko b", p=P)[:, :, bsl])
            prj = medp.tile([P, KD, BH], BF, tag="pre")
            st = ln_begin()

            def prj_out(m, ps, prj=prj, bvec=bvec, st=st):
                nc.scalar.activation(prj[:, m, :], ps, AF.Relu, bias=bvec[:, m:m + 1])
                ln_add(st, prj[:, m, :], m, KD)

            linear(wname, KD, KD,
                   rhs_fn=lambda k, x_in=x_in: x_in[:, k, :],
                   out_fn=prj_out)
            ln_apply(st, prj, KD, gvec, bevec, h_t[:, off:off + KD, :], DIM,
                     ng_t=ngvec)

        if hh == 0:
            nc.sync.dma_start(wg2, D["w_g2"].rearrange("(ko p) n -> p ko n", p=P))
            nc.sync.dma_start(wfc, D["w_fc"].rearrange("(ko p) n -> p ko n", p=P))

        # ---- two mamba blocks
        for l in range(2):
            # x' = silu(h @ w_inx + conv_b)  [P, KI, BH]  (native Silu table)
            xp = bigp.tile([P, KI, BH], BF, tag="xp")
            linear(f"w_inx{l}", KM, KI,
                   rhs_fn=lambda k, h_t=h_t: h_t[:, k, :],
                   out_fn=lambda m, ps, xp=xp, l=l: nc.scalar.activation(
                       xp[:, m, :], ps, AF.Silu, bias=b_inx[l][:, m:m + 1]))

            # x_proj m0 -> dblr; dt matmuls for chunks 0..15 interleaved into
            # the m1 (B/C) chain so the ACT softplus (exp/ln) drain never
            # stalls the PE.
            wx = wres.tile([P, KI, 192], BF, tag="wx")
            nc.sync.dma_start(wx, D[f"w_x{l}"].rearrange("(ko p) n -> p ko n", p=P))
            wdt = wres.tile([P, D_INNER], BF, tag="wdt")
            nc.sync.dma_start(wdt, D[f"w_dt{l}"][:, :])
            ps0 = psm.tile([P, BH], F32, tag="mm")
            for k in range(KI):
                nc.tensor.matmul(ps0, wx[:, k, 0:DT_RANK], xp[:, k, :],
                                 start=(k == 0), stop=(k == KI - 1))
            dblr = smc.tile([P, BH], BF, tag="dblr")
            nc.vector.tensor_copy(dblr, ps0)

            def dt_mm(dtg, i, k, wide=False):
                if wide and i % 2:
                    psd = pss.tile([P, BH], F32, tag="st")
                else:
                    psd = psd_p.tile([P, BH], F32, tag="psd")
                nc.tensor.matmul(psd, wdt[:, k * P:(k + 1) * P], dblr,
                                 start=True, stop=True)
                ex = sm.tile([P, BH], F32, tag="f32tmp")
                nc.scalar.activation(ex, psd, AF.Exp, bias=b_dt[l][:, k:k + 1])
                nc.scalar.activation(dtg[:, i, :], ex, AF.Ln, bias=ones_f)

            dtg1 = medp.tile([P, KM, BH], BF, tag="pre")
            ps1 = pss.tile([64, BH], F32, tag="st")
            for i in range(KM):
                dt_mm(dtg1, i, i)
                for kk in (2 * i, 2 * i + 1):
                    nc.tensor.matmul(ps1, wx[:, kk, 128:192], xp[:, kk, :],
                                     start=(kk == 0), stop=(kk == KI - 1))
            bcb = sm1.tile([D_STATE, BH], F32, tag="bcb")
            nc.scalar.copy(bcb, ps1[0:D_STATE])
            bcc = sm1.tile([D_STATE, BH], F32, tag="bcc")
            nc.scalar.copy(bcc, ps1[32:32 + D_STATE])
            prod = sm1.tile([D_STATE, BH], F32, tag="prod")
            nc.vector.tensor_tensor(prod, bcb, bcc, OP.mult)

            # fused y loop: y = x' * (dt*bc + D) * silu(z), native Silu table.
            wzr = D[f"w_inz{l}"].rearrange("(ko p) n -> p ko n", p=P)

            def z_part(k, kk_in_pair, wz_t):
                psz = psm.tile([P, BH], F32, tag="mm")
                for j in range(KM):
                    nc.tensor.matmul(psz, wz_t[:, j, kk_in_pair * P:(kk_in_pair + 1) * P],
                                     h_t[:, j, :],
                                     start=(j == 0), stop=(j == KM - 1))
                sz = szp.tile([P, BH], BF, tag="sz")
                nc.scalar.activation(sz, psz, AF.Silu, bias=bz[l][:, k:k + 1])
                return sz

            def y_part(k, dtg, i, sz):
                u = sm.tile([P, BH], F32, tag="f32tmp")
                nc.vector.tensor_tensor(u, dtg[:, i, :], BCt, OP.mult)
                nc.vector.scalar_tensor_tensor(u, u, dvec[l][:, k:k + 1],
                                               xp[:, k, :], OP.add, OP.mult)
                nc.vector.tensor_tensor(xp[:, k, :], u, sz, OP.mult)

            def wz_tile(k0):
                t = wt.tile([P, KM, 2 * P], BF, tag="wt")
                nc.sync.dma_start(t, wzr[:, :, k0 * P:(k0 + 2) * P])
                return t

            # first z pair runs while the tiny bc reduction completes
            t01 = wz_tile(0)
            sz01 = [z_part(kk, kk, t01) for kk in range(2)]
            ps_bc = pss.tile([1, BH], F32, tag="st")
            nc.tensor.matmul(ps_bc, ones_f[:D_STATE], prod, start=True, stop=True)
            bc1 = smc.tile([1, BH], F32, tag="r1")
            nc.scalar.copy(bc1, ps_bc)
            BCt = smc.tile([P, BH], F32, tag="bcast")
            nc.gpsimd.partition_broadcast(BCt, bc1)
            for kk in range(2):
                y_part(kk, dtg1, kk, sz01[kk])
            for k0 in range(2, KM, 2):           # rest of B1: chunks 2..15
                t = wz_tile(k0)
                for kk in range(2):
                    k = k0 + kk
                    y_part(k, dtg1, k, z_part(k, kk, t))
            # B2: dt for chunks 16..31 computed in 4-MM micro-blocks two
            # chunks ahead of use; 4 rotating banks (psd + idle stats slots)
            # mean the PE never waits on the ACT softplus drain.
            dtg2 = medp.tile([P, KM, BH], BF, tag="pre")
            wo_pref = None
            for k0 in range(KM, KI, 2):
                j = (k0 - KM) // 2
                if j % 2 == 0 and j < 8:
                    base = KM + 2 * j
                    for i in range(4):
                        dt_mm(dtg2, base - KM + i, base + i, wide=True)
                if k0 == KI - 4:
                    wo_pref = linear_prefetch(f"w_out{l}", KI, KM,
                                              mtile=1, ktile=KI, groups=2)
                t = wz_tile(k0)
                for kk in range(2):
                    k = k0 + kk
                    y_part(k, dtg2, k - KM, z_part(k, kk, t))

            # out_proj + residual -> pre (stats inline), then LN -> h
            pre = medp.tile([P, KM, BH], BF, tag="pre")
            st = ln_begin()

            def pre_out(m, ps, pre=pre, h_t=h_t, st=st, l=l):
                nc.vector.scalar_tensor_tensor(pre[:, m, :], h_t[:, m, :],
                                               bres[l][:, m:m + 1], ps,
                                               OP.add, OP.add)
                ln_add(st, pre[:, m, :], m, KM)

            linear(f"w_out{l}", KI, KM,
                   rhs_fn=lambda k, xp=xp: xp[:, k, :],
                   out_fn=pre_out, mtile=1, ktile=KI, prefetched=wo_pref)
            ln_apply(st, pre, KM, g_ln[l], b_ln[l], h_t, D_MODEL, ng_t=ng_ln[l])

        # ---- gate head (folded attn) -> gate weights [BH, 2]
        gh = medp.tile([P, KD, BH], BF, tag="pre")
        linear("w_gate", KM, KD,
               rhs_fn=lambda k, h_t=h_t: h_t[:, k, :],
               out_fn=lambda m, ps, gh=gh: nc.scalar.activation(
                   gh[:, m, :], ps, AF.Relu, bias=b_gate[:, m:m + 1]))
        ps_lg = pss.tile([2, BH], F32, tag="st")
        for k in range(KD):
            nc.tensor.matmul(ps_lg, wg2[:, k, :], gh[:, k, :],
                             start=(k == 0), stop=(k == KD - 1))
        lg = sm1.tile([2, BH], F32, tag="lg")
        nc.scalar.activation(lg, ps_lg, AF.Identity, bias=b_g2)
        ps_d = pss.tile([1, BH], F32, tag="st")
        nc.tensor.matmul(ps_d, pm1, lg, start=True, stop=True)
        # sigmoid via exp: gw0 = 1/(1+exp(-dd))
        egw = smc.tile([1, BH], F32, tag="r1")
        nc.scalar.activation(egw, ps_d, AF.Exp, scale=-1.0)
        nc.vector.tensor_scalar(egw, egw, 1.0, None, OP.add)
        gw0 = smc.tile([1, BH], F32, tag="r1")
        nc.vector.reciprocal(gw0, egw)
        gw1 = smc.tile([1, BH], F32, tag="r1")
        nc.vector.tensor_scalar(gw1, gw0, -1.0, 1.0, OP.mult, OP.add)
        nc.sync.dma_start(D["gw"][bsl, 0:1].rearrange("b t -> t b"), gw0)
        nc.sync.dma_start(D["gw"][bsl, 1:2].rearrange("b t -> t b"), gw1)

        # ---- fc head + final LN -> fused [BH, 256]
        pf = sm1.tile([P, KO, BH], F32, tag="pf")
        st = ln_begin()
        for m in range(KO):
            ps = psm.tile([P, BH], F32, tag="mm")
            for k in range(KM):
                nc.tensor.matmul(ps, wfc[:, k, m * P:(m + 1) * P], h_t[:, k, :],
                                 start=(k == 0), stop=(k == KM - 1))
            nc.scalar.activation(pf[:, m, :], ps, AF.Identity, bias=b_fc[:, m:m + 1])
            ln_add(st, pf[:, m, :], m, KO, fp32=True)
        ff = sm1.tile([P, KO, BH], F32, tag="ff")
        ln_apply(st, pf, KO, g_fin, b_fin, ff, OUT)
        for b4 in range(BH // P):
            ob = sm2.tile([P, OUT], F32, tag="ob")
            for m in range(KO):
                p_t = pst.tile([P, P], F32, tag="tp")
                nc.tensor.transpose(p_t, ff[:, m, b4 * P:(b4 + 1) * P], ident)
                nc.vector.tensor_copy(ob[:, m * P:(m + 1) * P], p_t)
            nc.sync.dma_start(D["fused"][hh * BH + b4 * P: hh * BH + (b4 + 1) * P, :], ob)


# ---------------------------------------------------------------- build + run

_CACHE = {}
_LOCK = threading.Lock()


def _get_program():
    with _LOCK:
        if "nc" in _CACHE:
            return _CACHE["nc"]
        nc = bacc.Bacc("TRN2", target_bir_lowering=False, debug=False,
                       num_devices=N_CORES)
        D = {}

        def din(name, shape, dt):
            D[name] = nc.dram_tensor(name, shape, dt, kind="ExternalInput").ap()

        din("imgT", (DIM, BC), BF)
        din("txtT", (DIM, BC), BF)
        din("w_img", (DIM, DIM), BF)
        din("w_txt", (DIM, DIM), BF)
        for n in ("b_img", "g_img", "be_img", "b_txt", "g_txt", "be_txt", "b_gate"):
            din(n, (P, KD), F32)
        for l in range(2):
            din(f"w_inx{l}", (D_MODEL, D_INNER), BF)
            din(f"w_inz{l}", (D_MODEL, D_INNER), BF)
            din(f"w_x{l}", (D_INNER, 192), BF)
            din(f"w_dt{l}", (DT_RANK, D_INNER), BF)
            din(f"w_out{l}", (D_INNER, D_MODEL), BF)
            for n in (f"b_inx{l}", f"b_dt{l}", f"dvec{l}", f"bz{l}"):
                din(n, (P, KI), F32)
            for n in (f"g_ln{l}", f"b_ln{l}", f"bres{l}"):
                din(n, (P, KM), F32)
        din("w_gate", (D_MODEL, DIM), BF)
        din("w_g2", (DIM, 2), BF)
        din("b_g2", (2, 1), F32)
        din("pm1", (2, 1), F32)
        din("w_fc", (D_MODEL, OUT), BF)
        for n in ("b_fc", "g_fin", "b_fin"):
            din(n, (P, KO), F32)
        D["fused"] = nc.dram_tensor("fused", (BC, OUT), F32, kind="ExternalOutput").ap()
        D["gw"] = nc.dram_tensor("gw", (BC, 2), F32, kind="ExternalOutput").ap()

        from contextlib import ExitStack
        with tile.TileContext(nc) as tc:
            with ExitStack() as es:
                _emit(es, nc, tc, D)
        nc.compile()
        _CACHE["nc"] = nc
        return nc


def _install_ntff_hook():
    """trace=True under axon needs antenv.axon_hooks, absent in this image."""
    if "antenv.axon_hooks" in sys.modules:
        return
    try:
        from trn_agent_boot.trn_boot import _ntff_profile_via_ctypes
        hook = _ntff_profile_via_ctypes("/opt/axon/libaxon_pjrt.so")
    except Exception:
        hook = None
    mod = types.ModuleType("antenv.axon_hooks")
    mod.get_axon_ntff_profile_hook = lambda: hook
    sys.modules["antenv.axon_hooks"] = mod


def kernel(_trace=False, **inputs):
    from concourse.bass_utils import run_bass_kernel_spmd

    _install_ntff_hook()
    nc = _get_program()
    w = _prep_weights(inputs)
    imgT = np.ascontiguousarray(
        np.asarray(inputs["image_features"], np.float32).astype(BF16NP).T)  # [DIM, B]
    txtT = np.ascontiguousarray(
        np.asarray(inputs["text_features"], np.float32).astype(BF16NP).T)
    in_maps = []
    for c in range(N_CORES):
        m = dict(w)
        m["imgT"] = np.ascontiguousarray(imgT[:, c * BC:(c + 1) * BC])
        m["txtT"] = np.ascontiguousarray(txtT[:, c * BC:(c + 1) * BC])
        in_maps.append(m)
    res = run_bass_kernel_spmd(nc, in_maps, core_ids=list(range(N_CORES)),
                               trace=_trace)
    fused = np.concatenate([res.results[c]["fused"] for c in range(N_CORES)], axis=0)
    gw = np.concatenate([res.results[c]["gw"] for c in range(N_CORES)], axis=0)
    if _trace:
        kernel.last_exec_time_ns = res.exec_time_ns
        kernel.last_results = res
    return fused, gw
